# revision 1
# baseline (speedup 1.0000x reference)
"""MoE-attention kernel for 8 Trainium2 NeuronCores.

Sharding: token-parallel. Core c handles sequence b = c//2, query-token half
half = c%2 (512 query tokens). Each core computes all 20 experts for its
query tokens; K/V context is the full 1024-token sequence, fed with the
local half FIRST (attention is permutation-invariant over key positions).
No collectives: out_proj partial sums are avoided by giving every core the
full feature dim for its own query tokens.

v3 layout strategy (bf16 matmuls, f32 PSUM accumulate):
  xT      pre-transposed on host, loaded as two tiles (dt 0-4 / 5-9)
  all weights DMA'd up-front in a few large transfers (no per-pair DMAs)
  kT      [128(2 experts), 1024t]  single N=1024 chain (2-bank PSUM tile)
  scoresT [128kt, 2, 512qt]  2 key-tiles per PSUM tile -> one big exp
  attnT   = exp(scoresT*scale)  (no max-subtraction: |scores*scale| < ~3)
  software pipeline: PROJ(p+1) emitted before ATT(p), so the projection
  copies always hide under the previous pair's attention; within ATT,
  exp(unit u) on Act overlaps eo-matmuls(u-1) on PE
  v       [128t, kt, e, 65]  natural layout + ones column -> sumexp row
  eoT     [65h, 512qt] = v.T @ attnT ; row 64 = sumexp
  combT   10 per-ht tiles [128h', 512qt] = eoT * gate/sumexp
  out     [512t, 1280] = combT.T @ out_w + out_b (N=1024 + N=256 chunks,
          stationary combT[ht] reused across the two chunks)
"""

import numpy as np

import concourse.bass as bass
import concourse.mybir as mybir
import concourse.tile as tile
from concourse import bacc
from concourse.bass_utils import run_bass_kernel_spmd

F32 = mybir.dt.float32
BF16 = mybir.dt.bfloat16
MM_DT = BF16
try:
    import ml_dtypes as _mld
    NP_MM = np.dtype(_mld.bfloat16)
except Exception as e:  # pragma: no cover
    raise RuntimeError("ml_dtypes required for bf16 host packing") from e
AF = mybir.ActivationFunctionType

B = 4
S = 1024          # sequence length (full context per core)
D = 1280          # d_model
E = 20            # experts
EP = E // 2       # expert pairs
H = 64            # head dim
SL = 512          # local query tokens per core
DT = D // 128     # 10 d-tiles
HT = D // 128     # 10 h'-tiles
KT = S // 128     # 8 key tiles
SCALE = float(H) ** -0.5
NCORES = 8
GROUPS = [(0, 8), (8, 16), (16, 20)]
GROUP_OF_PAIR = {p: gi for gi, (g0, g1) in enumerate(GROUPS)
                 for p in range(g0 // 2, g1 // 2)}
PAIR_STARTS_GROUP = {g0 // 2: gi for gi, (g0, g1) in enumerate(GROUPS)}


def _mm(nc, out, lhsT, rhs, **kw):
    nc.tensor.matmul(out, lhsT, rhs, **kw)


def _emit(tc, xT_d, wqk_d, bqk_d, wv_d, bv_d, rw_d, rb_d, ow_d, ob_d, out_d):
    nc = tc.nc
    with (
        tc.tile_pool(name="const", bufs=1) as const,
        tc.tile_pool(name="io", bufs=2) as io,
        tc.tile_pool(name="vp", bufs=2) as vp,
        tc.tile_pool(name="qk", bufs=2) as qkp,
        tc.tile_pool(name="at", bufs=4) as atp,
        tc.tile_pool(name="sm", bufs=2) as smp,
        tc.tile_pool(name="ps", bufs=3, space="PSUM") as psp,
        tc.tile_pool(name="pe", bufs=2, space="PSUM") as pep,
    ):
        # ---- constants + all weights up-front (few large DMAs) ----
        ones_f32 = const.tile([128, 512], F32, name="ones_f32")
        nc.gpsimd.memset(ones_f32, 1.0)
        ones_row = const.tile([1, S], MM_DT, name="ones_row")
        nc.vector.tensor_copy(ones_row[:, 0:512], ones_f32[0:1, :])
        nc.vector.tensor_copy(ones_row[:, 512:1024], ones_f32[0:1, :])
        ones20 = const.tile([E, 1], MM_DT, name="ones20")
        nc.vector.tensor_copy(ones20, ones_f32[0:E, 0:1])

        # router weights first (tiny), then x^T as three tiles so the router
        # chain starts as soon as the first two d-tiles land
        rw_sb = const.tile([128, DT, E], MM_DT, name="rw_sb")
        nc.sync.dma_start(out=rw_sb, in_=rw_d.rearrange("(t p) e -> p t e", p=128))
        rb_sb = const.tile([1, E], MM_DT, name="rb_sb")
        nc.sync.dma_start(out=rb_sb, in_=rb_d[None, :])

        xTa1 = const.tile([128, 2, S], MM_DT, name="xTa1")
        xTa2 = const.tile([128, 3, S], MM_DT, name="xTa2")
        xTb = const.tile([128, DT - 5, S], MM_DT, name="xTb")
        xin = xT_d.rearrange("(t p) s -> p t s", p=128)
        nc.sync.dma_start(out=xTa1, in_=xin[:, 0:2, :])
        nc.scalar.dma_start(out=xTb, in_=xin[:, 5:, :])
        nc.sync.dma_start(out=xTa2, in_=xin[:, 2:5, :])

        def xT(dt):
            if dt < 2:
                return xTa1[:, dt, :]
            if dt < 5:
                return xTa2[:, dt - 2, :]
            return xTb[:, dt - 5, :]

        wv_sb = const.tile([128, DT, E * H], MM_DT, name="wv_sb")
        wv_in = wv_d.rearrange("(t p) h -> p t h", p=128)
        nc.sync.dma_start(out=wv_sb[:, :, 0:8 * H], in_=wv_in[:, :, 0:8 * H])
        nc.scalar.dma_start(out=wv_sb[:, :, 8 * H:], in_=wv_in[:, :, 8 * H:])
        bv_row = const.tile([1, E * H], MM_DT, name="bv_row")
        nc.sync.dma_start(out=bv_row, in_=bv_d)

        wqk_sb = const.tile([128, EP, DT, 256], MM_DT, name="wqk_sb")
        wqk_in = wqk_d.rearrange("e (t p) h -> p e t h", p=128)
        nc.sync.dma_start(out=wqk_sb[:, 0:EP // 2], in_=wqk_in[:, 0:EP // 2])
        nc.scalar.dma_start(out=wqk_sb[:, EP // 2:], in_=wqk_in[:, EP // 2:])
        bqk_sb = const.tile([1, EP, 256], MM_DT, name="bqk_sb")
        nc.sync.dma_start(out=bqk_sb, in_=bqk_d[None, :, :])

        ow_sb = const.tile([128, DT, D], MM_DT, name="ow_sb")
        ow_in = ow_d.rearrange("(t p) n -> p t n", p=128)
        nc.sync.dma_start(out=ow_sb[:, 0:DT // 2], in_=ow_in[:, 0:DT // 2])
        nc.scalar.dma_start(out=ow_sb[:, DT // 2:], in_=ow_in[:, DT // 2:])
        ob_sb = const.tile([1, D], MM_DT, name="ob_sb")
        nc.sync.dma_start(out=ob_sb, in_=ob_d[None, :])

        # per-ht combined-feature tiles (separate so out_proj's ht chain only
        # waits on the normalize that wrote that ht)
        combT = [const.tile([128, SL], MM_DT, name=f"combT{h}")
                 for h in range(HT)]

        # ---- router logits (gates finished after group-0 V so the PE
        # never waits on the Act queue's DMA issues) ----
        exp_router = const.tile([E, SL], MM_DT, name="exp_router")
        gates_sb = const.tile([E, SL], F32, name="gates_sb")
        inv_rsum = const.tile([1, SL], F32, name="inv_rsum")
        inv_rep = const.tile([E, SL], F32, name="inv_rep")

        rt_ps = psp.tile([E, SL], F32, name="rt_ps", tag="sc")
        for dt in range(DT):
            _mm(nc, rt_ps, rw_sb[:, dt, :], xT(dt)[:, 0:SL],
                start=(dt == 0), stop=False)
        _mm(nc, rt_ps, rb_sb, ones_row[:, 0:SL], start=False, stop=True)
        nc.scalar.activation(exp_router, rt_ps, AF.Exp)

        def finish_gates():
            rs_ps = psp.tile([1, SL], F32, name="rs_ps", tag="sc")
            _mm(nc, rs_ps, ones20, exp_router, start=True, stop=True)
            nc.vector.reciprocal(inv_rsum, rs_ps)
            nc.gpsimd.partition_broadcast(inv_rep, inv_rsum)
            nc.vector.tensor_mul(gates_sb, exp_router, inv_rep)

        v_tiles = {}

        def emit_vloop(gi):
            g0, g1 = GROUPS[gi]
            gsz = g1 - g0
            v_sb = vp.tile([128, KT, gsz, H + 1], MM_DT, name="v_sb", tag="vg")
            v_tiles[gi] = v_sb
            nc.vector.tensor_copy(
                v_sb[:, :, :, H],
                ones_f32[:, 0:KT * gsz].rearrange("p (a b) -> p a b", a=KT),
            )
            for tt in range(KT):
                v_ps = psp.tile([128, gsz * H], F32, name="v_ps", tag="sc")
                for dt in range(DT):
                    _mm(nc, v_ps, xT(dt)[:, tt * 128:(tt + 1) * 128],
                        wv_sb[:, dt, g0 * H:g1 * H],
                        start=(dt == 0), stop=False)
                _mm(nc, v_ps, ones_row[:, 0:128], bv_row[:, g0 * H:g1 * H],
                    start=False, stop=True)
                nc.vector.tensor_copy(
                    v_sb[:, tt, :, 0:H],
                    v_ps.rearrange("p (e h) -> p e h", e=gsz),
                )

        def emit_qt(p, sink):
            qt_ps = psp.tile([128, SL], F32, name="qt_ps", tag="sc")
            for dt in range(DT):
                _mm(nc, qt_ps, wqk_sb[:, p, dt, 0:128], xT(dt)[:, 0:SL],
                    start=(dt == 0), stop=False)
            _mm(nc, qt_ps, bqk_sb[:, p, 0:128], ones_row[:, 0:SL],
                start=False, stop=True)
            q_sb = qkp.tile([128, SL], MM_DT, name="q_sb", tag="q")
            nc.vector.tensor_copy(q_sb, qt_ps)
            sink[0] = q_sb

        def emit_kt(p, ch, sink):
            sl = slice(ch * 512, (ch + 1) * 512)
            kt_ps = psp.tile([128, 512], F32, name=f"kt_ps{ch}", tag="sc")
            for dt in range(DT):
                _mm(nc, kt_ps, wqk_sb[:, p, dt, 128:256], xT(dt)[:, sl],
                    start=(dt == 0), stop=False)
            _mm(nc, kt_ps, bqk_sb[:, p, 128:256], ones_row[:, 0:512],
                start=False, stop=True)
            k_sb = qkp.tile([128, 512], MM_DT, name=f"k_sb{ch}", tag=f"k{ch}")
            nc.vector.tensor_copy(k_sb, kt_ps)
            sink[1][ch] = k_sb

        def emit_proj(p):
            sink = [None, [None, None]]
            emit_qt(p, sink)
            emit_kt(p, 0, sink)
            emit_kt(p, 1, sink)
            return sink

        def emit_att(p, sink, fillers=()):
            q_sb, k_chunks = sink[0], sink[1]
            gi = GROUP_OF_PAIR[p]
            g0 = GROUPS[gi][0]
            v_sb = v_tiles[gi]
            g0_tiles = []
            for sub in range(2):
                g_row0 = smp.tile([1, SL], F32, name="g_row0")
                nc.sync.dma_start(
                    out=g_row0, in_=gates_sb[2 * p + sub:2 * p + sub + 1, :])
                g0_tiles.append(g_row0)

            eo_tiles = [
                pep.tile([H + 1, SL], F32, name=f"eo_ps{s}", tag="eo")
                for s in range(2)
            ]

            def emit_eo(ats, c):
                for sub in range(2):
                    i = 2 * p + sub - g0
                    for j in range(2):
                        _mm(nc, eo_tiles[sub], v_sb[:, 2 * c + j, i, :],
                            ats[sub][:, j, :],
                            start=(c == 0 and j == 0),
                            stop=(c == KT // 2 - 1 and j == 1))

            # scores: alternate sub-expert row groups (rows 0-63 / 64-127)
            # on consecutive matmuls so disjoint row tiles overlap in the PE
            pending = []
            for c in range(KT // 2):
                scs = [psp.tile([128, 2, SL], F32, name=f"sc2_{s}", tag="sc")
                       for s in range(2)]
                for j in range(2):
                    kt = 2 * c + j
                    ksl = slice((kt % 4) * 128, (kt % 4 + 1) * 128)
                    for sub in range(2):
                        po = sub * 64
                        _mm(nc, scs[sub][:, j, :],
                            k_chunks[kt // 4][po:po + 64, ksl],
                            q_sb[po:po + 64, :], start=True, stop=True)
                ats = []
                for sub in range(2):
                    at2 = atp.tile([128, 2, SL], MM_DT, name="at2", tag="at")
                    nc.scalar.activation(at2, scs[sub], AF.Exp, scale=SCALE)
                    ats.append(at2)
                pending.append((ats, c))
                if len(pending) > 1:
                    emit_eo(*pending.pop(0))
                if c < len(fillers):
                    fillers[c]()
            emit_eo(*pending.pop(0))

            # gate/sumexp normalization of eoT -> combT[p]
            for sub in range(2):
                eo_ps = eo_tiles[sub]
                s_inv = smp.tile([1, SL], F32, name="s_inv")
                nc.vector.reciprocal(s_inv, eo_ps[H:H + 1, :])
                g_row = smp.tile([1, SL], F32, name="g_row")
                nc.vector.tensor_mul(g_row, s_inv, g0_tiles[sub])
                sc64 = smp.tile([H, SL], F32, name="sc64")
                nc.gpsimd.partition_broadcast(sc64, g_row)
                nc.vector.tensor_mul(
                    combT[p][sub * 64:sub * 64 + 64, :], eo_ps[0:H, :], sc64
                )

        # ---- pipelined schedule: PROJ(p+1) lands before ATT(p) so its
        # q/k copies hide under the previous pair's attention ----
        emit_vloop(0)
        finish_gates()
        proj = {0: emit_proj(0)}
        for p in range(EP):
            nxt = p + 1
            if nxt in PAIR_STARTS_GROUP:
                emit_vloop(PAIR_STARTS_GROUP[nxt])
            if nxt < EP:
                proj[nxt] = emit_proj(nxt)
            emit_att(p, proj.pop(p))

        # ---- out projection: for each token tile, one N=1024 chunk and one
        # N=256 chunk, sharing each combT[ht] stationary back-to-back ----
        OCH = [(0, 512), (512, 1024), (1024, 1280)]
        last_tt = SL // 128 - 1
        for tt in range(SL // 128):
            tsl = slice(tt * 128, (tt + 1) * 128)
            o_sb = io.tile([128, D], F32, name="o_sb", tag="o_sb")
            for ci, (c0, c1) in enumerate(OCH):
                op_ps = psp.tile([128, c1 - c0], F32, name=f"op{ci}", tag="sc")
                for ht in range(HT):
                    _mm(nc, op_ps, combT[ht][:, tsl], ow_sb[:, ht, c0:c1],
                        start=(ht == 0), stop=False)
                _mm(nc, op_ps, ones_row[:, 0:128], ob_sb[:, c0:c1],
                    start=False, stop=True)
                nc.vector.tensor_copy(o_sb[:, c0:c1], op_ps)
                if tt == last_tt:
                    # per-chunk stores so the kernel tail is only the last
                    # 256-wide chunk's copy + DMA
                    nc.sync.dma_start(out=out_d[tsl, c0:c1],
                                      in_=o_sb[:, c0:c1])
            if tt != last_tt:
                nc.sync.dma_start(out=out_d[tsl, :], in_=o_sb)


def declare_tensors(nc):
    xT_d = nc.dram_tensor("xT", [D, S], MM_DT, kind="ExternalInput").ap()
    wqk_d = nc.dram_tensor("wqk", [EP, D, 256], MM_DT, kind="ExternalInput").ap()
    bqk_d = nc.dram_tensor("bqk", [EP, 256], MM_DT, kind="ExternalInput").ap()
    wv_d = nc.dram_tensor("wv", [D, E * H], MM_DT, kind="ExternalInput").ap()
    bv_d = nc.dram_tensor("bv", [1, E * H], MM_DT, kind="ExternalInput").ap()
    rw_d = nc.dram_tensor("router_w", [D, E], MM_DT, kind="ExternalInput").ap()
    rb_d = nc.dram_tensor("router_b", [E], MM_DT, kind="ExternalInput").ap()
    ow_d = nc.dram_tensor("out_w", [D, D], MM_DT, kind="ExternalInput").ap()
    ob_d = nc.dram_tensor("out_b", [D], MM_DT, kind="ExternalInput").ap()
    out_d = nc.dram_tensor("out", [SL, D], F32, kind="ExternalOutput").ap()
    return (xT_d, wqk_d, bqk_d, wv_d, bv_d, rw_d, rb_d, ow_d, ob_d, out_d)


def build_nc():
    nc = bacc.Bacc("TRN2", target_bir_lowering=False, debug=False,
                   num_devices=NCORES)
    tensors = declare_tensors(nc)
    with tile.TileContext(nc) as tc:
        _emit(tc, *tensors)
    nc.compile()
    return nc


_NC = None


def _get_nc():
    global _NC
    if _NC is None:
        _NC = build_nc()
    return _NC


def make_in_maps(x, wqkv, bqkv, router_w, router_b, out_w, out_b):
    x = np.ascontiguousarray(np.asarray(x, np.float32))
    wqkv = np.asarray(wqkv, np.float32)
    bqkv = np.asarray(bqkv, np.float32)
    wq = wqkv[:, :, 0:H].reshape(EP, 2, D, H).transpose(0, 2, 1, 3).reshape(EP, D, 128)
    wk = wqkv[:, :, H:2 * H].reshape(EP, 2, D, H).transpose(0, 2, 1, 3).reshape(EP, D, 128)
    wqk = np.concatenate([wq, wk], axis=-1)
    bq = bqkv[:, 0:H].reshape(EP, 128)
    bk = bqkv[:, H:2 * H].reshape(EP, 128)
    bqk = np.concatenate([bq, bk], axis=-1)
    wv = wqkv[:, :, 2 * H:3 * H].transpose(1, 0, 2).reshape(D, E * H)
    bv = bqkv[:, 2 * H:3 * H].reshape(1, E * H)

    def _c(a):
        return np.ascontiguousarray(np.asarray(a, np.float32).astype(NP_MM))
    shared = {
        "wqk": _c(wqk), "bqk": _c(bqk), "wv": _c(wv), "bv": _c(bv),
        "router_w": _c(router_w), "router_b": _c(router_b),
        "out_w": _c(out_w), "out_b": _c(out_b),
    }
    in_maps = []
    for c in range(NCORES):
        b, half = c // 2, c % 2
        xb = x[b]
        if half == 0:
            x_ctx = xb
        else:
            x_ctx = np.concatenate([xb[SL:], xb[:SL]], axis=0)
        in_maps.append({"xT": _c(x_ctx.T), **shared})
    return in_maps


def gather_out(results):
    out = np.empty((B, S, D), np.float32)
    for c in range(NCORES):
        b, half = c // 2, c % 2
        out[b, half * SL:(half + 1) * SL] = results[c]["out"]
    return out


def kernel(x, wqkv, bqkv, router_w, router_b, out_w, out_b):
    nc = _get_nc()
    in_maps = make_in_maps(x, wqkv, bqkv, router_w, router_b, out_w, out_b)
    res = run_bass_kernel_spmd(nc, in_maps, core_ids=list(range(NCORES)))
    return gather_out(res.results)



# revision 4
# speedup vs baseline: 1.2820x; 1.2820x over previous
"""MoE-attention kernel for 8 Trainium2 NeuronCores.

Sharding: token-parallel. Core c handles sequence b = c//2, query-token half
half = c%2 (512 query tokens). Each core computes all 20 experts for its
query tokens; K/V context is the full 1024-token sequence, fed with the
local half FIRST (attention is permutation-invariant over key positions).
No collectives: out_proj partial sums are avoided by giving every core the
full feature dim for its own query tokens.

v4 layout strategy (fp8 DoubleRow matmuls for projections + attn*v):
  x, wv, wqk, rw  host-packed fp8e4m3 as [128, 5, 2, N]: contraction d-dim
                  split into 5 chunks of 256 = (2 planes x 128 partitions),
                  consumed by DoubleRow matmuls (measured ~1.9x bf16 FLOP/s)
  biases          bf16, added via regular K=1 matmuls into the same PSUM group
  q/k             PSUM f32 -> SBUF bf16 (scores stay bf16: contraction is only
                  64, served by PE quadrant pairs at full rate; fp8 DR cannot
                  beat that shape)
  scoresT [128kt, 2, 512qt]  2 key-tiles per PSUM tile -> one big exp
  attnT = exp(scoresT*scale) emitted as fp8 (no max-subtraction: scores are
                  bounded, exp < ~50 fits fp8e4m3 range 448)
  v       [128t, kt, e, 65] fp8 with ones column -> sumexp row
  eoT     [65h, 512qt] = DoubleRow over (v 2-key-tile planes, attn planes)
  combT   10 per-ht tiles [128h', 512qt] = eoT * gate/sumexp   (bf16)
  out     [512t, 1280] = combT.T @ out_w + out_b, all bf16 (fp8 out_proj
                  fails the accuracy budget; measured numerically)
  software pipeline: PROJ(p+1) emitted before ATT(p), so the projection
  copies always hide under the previous pair's attention; within ATT,
  exp(unit u) on Act overlaps eo-matmuls(u-1) on PE
"""

import numpy as np

import concourse.bass as bass
import concourse.mybir as mybir
import concourse.tile as tile
from concourse import bacc
from concourse.bass_utils import run_bass_kernel_spmd

F32 = mybir.dt.float32
BF16 = mybir.dt.bfloat16
FP8 = mybir.dt.float8e4
DR = mybir.MatmulPerfMode.DoubleRow
MM_DT = BF16
try:
    import ml_dtypes as _mld
    NP_MM = np.dtype(_mld.bfloat16)
    NP_F8 = np.dtype(_mld.float8_e4m3)
except Exception as e:  # pragma: no cover
    raise RuntimeError("ml_dtypes required for bf16/fp8 host packing") from e
AF = mybir.ActivationFunctionType

B = 4
S = 1024          # sequence length (full context per core)
D = 1280          # d_model
E = 20            # experts
EP = E // 2       # expert pairs
H = 64            # head dim
SL = 512          # local query tokens per core
DC = 5            # d-model DoubleRow chunks (5 x (2 x 128) = 1280)
DT = D // 128     # 10 d-tiles (out_proj contraction)
HT = D // 128     # 10 h'-tiles
KT = S // 128     # 8 key tiles
SCALE = float(H) ** -0.5
NCORES = 8
GROUPS = [(0, 8), (8, 16), (16, 20)]
GROUP_OF_PAIR = {p: gi for gi, (g0, g1) in enumerate(GROUPS)
                 for p in range(g0 // 2, g1 // 2)}
PAIR_STARTS_GROUP = {g0 // 2: gi for gi, (g0, g1) in enumerate(GROUPS)}


def _mm(nc, out, lhsT, rhs, **kw):
    nc.tensor.matmul(out, lhsT, rhs, **kw)


def _emit(tc, xpk_d, wqk_d, bqk_d, wv_d, bv_d, rw_d, rb_d, ow_d, ob_d, out_d):
    nc = tc.nc
    with (
        tc.tile_pool(name="const", bufs=1) as const,
        tc.tile_pool(name="io", bufs=2) as io,
        tc.tile_pool(name="vp", bufs=2) as vp,
        tc.tile_pool(name="qk", bufs=2) as qkp,
        tc.tile_pool(name="at", bufs=4) as atp,
        tc.tile_pool(name="sm", bufs=2) as smp,
        tc.tile_pool(name="ps", bufs=3, space="PSUM") as psp,
        tc.tile_pool(name="pe", bufs=2, space="PSUM") as pep,
    ):
        # ---- constants + all weights up-front (few large DMAs) ----
        ones_f32 = const.tile([128, 512], F32, name="ones_f32")
        nc.gpsimd.memset(ones_f32, 1.0)
        ones_row = const.tile([1, S], MM_DT, name="ones_row")
        nc.vector.tensor_copy(ones_row[:, 0:512], ones_f32[0:1, :])
        nc.vector.tensor_copy(ones_row[:, 512:1024], ones_f32[0:1, :])
        ones20 = const.tile([E, 1], MM_DT, name="ones20")
        nc.vector.tensor_copy(ones20, ones_f32[0:E, 0:1])

        # router weights first (tiny), then x packed as three c-chunks so the
        # router chain starts as soon as the first chunk lands
        # router weights padded to 64 output cols: DoubleRow with M=20
        # (tile col 32) fails the walrus ISA check; M=64 passes
        rw_sb = const.tile([128, DC, 2, 64], FP8, name="rw_sb")
        nc.sync.dma_start(out=rw_sb, in_=rw_d)
        rb_sb = const.tile([1, 64], MM_DT, name="rb_sb")
        nc.sync.dma_start(out=rb_sb, in_=rb_d[None, :])

        xpk = const.tile([128, DC, 2, S], FP8, name="xpk")
        nc.sync.dma_start(out=xpk[:, 0:1], in_=xpk_d[:, 0:1])
        nc.scalar.dma_start(out=xpk[:, 1:3], in_=xpk_d[:, 1:3])
        nc.sync.dma_start(out=xpk[:, 3:5], in_=xpk_d[:, 3:5])

        wv_sb = const.tile([128, DC, 2, E * H], FP8, name="wv_sb")
        nc.sync.dma_start(out=wv_sb[:, :, :, 0:8 * H], in_=wv_d[:, :, :, 0:8 * H])
        nc.scalar.dma_start(out=wv_sb[:, :, :, 8 * H:], in_=wv_d[:, :, :, 8 * H:])
        bv_row = const.tile([1, E * H], MM_DT, name="bv_row")
        nc.sync.dma_start(out=bv_row, in_=bv_d)

        wqk_sb = const.tile([128, EP, DC, 2, 256], FP8, name="wqk_sb")
        nc.sync.dma_start(out=wqk_sb[:, 0:EP // 2], in_=wqk_d[:, 0:EP // 2])
        nc.scalar.dma_start(out=wqk_sb[:, EP // 2:], in_=wqk_d[:, EP // 2:])
        bqk_sb = const.tile([1, EP, 256], MM_DT, name="bqk_sb")
        nc.sync.dma_start(out=bqk_sb, in_=bqk_d[None, :, :])

        ow_sb = const.tile([128, DT, D], MM_DT, name="ow_sb")
        ow_in = ow_d.rearrange("(t p) n -> p t n", p=128)
        nc.sync.dma_start(out=ow_sb[:, 0:DT // 2], in_=ow_in[:, 0:DT // 2])
        nc.scalar.dma_start(out=ow_sb[:, DT // 2:], in_=ow_in[:, DT // 2:])
        ob_sb = const.tile([1, D], MM_DT, name="ob_sb")
        nc.sync.dma_start(out=ob_sb, in_=ob_d[None, :])

        # per-ht combined-feature tiles (separate so out_proj's ht chain only
        # waits on the normalize that wrote that ht)
        combT = [const.tile([128, SL], MM_DT, name=f"combT{h}")
                 for h in range(HT)]

        # ---- router logits (gates finished after group-0 V so the PE
        # never waits on the Act queue's DMA issues) ----
        exp_router = const.tile([E, SL], MM_DT, name="exp_router")
        gates_sb = const.tile([E, SL], F32, name="gates_sb")
        inv_rsum = const.tile([1, SL], F32, name="inv_rsum")
        inv_rep = const.tile([E, SL], F32, name="inv_rep")

        rt_ps = psp.tile([64, SL], F32, name="rt_ps", tag="sc")
        for c in range(DC):
            _mm(nc, rt_ps, rw_sb[:, c], xpk[:, c, :, 0:SL],
                start=(c == 0), stop=False, perf_mode=DR)
        _mm(nc, rt_ps, rb_sb, ones_row[:, 0:SL], start=False, stop=True)
        nc.scalar.activation(exp_router, rt_ps[0:E, :], AF.Exp)

        def finish_gates():
            rs_ps = psp.tile([1, SL], F32, name="rs_ps", tag="sc")
            _mm(nc, rs_ps, ones20, exp_router, start=True, stop=True)
            nc.vector.reciprocal(inv_rsum, rs_ps)
            nc.gpsimd.partition_broadcast(inv_rep, inv_rsum)
            nc.vector.tensor_mul(gates_sb, exp_router, inv_rep)

        v_tiles = {}

        def emit_vloop(gi):
            g0, g1 = GROUPS[gi]
            gsz = g1 - g0
            # row width padded 65 -> 68: dual-fp8 Ldweights requires the
            # plane (kt) byte-stride to be a multiple of 16 (gsz*68 = 544/272)
            v_sb = vp.tile([128, KT, gsz, 68], FP8, name="v_sb", tag="vg")
            v_tiles[gi] = v_sb
            nc.vector.tensor_copy(
                v_sb[:, :, :, H],
                ones_f32[:, 0:KT * gsz].rearrange("p (a b) -> p a b", a=KT),
            )
            for tt in range(KT):
                v_ps = psp.tile([128, gsz * H], F32, name="v_ps", tag="sc")
                for c in range(DC):
                    _mm(nc, v_ps, xpk[:, c, :, tt * 128:(tt + 1) * 128],
                        wv_sb[:, c, :, g0 * H:g1 * H],
                        start=(c == 0), stop=False, perf_mode=DR)
                _mm(nc, v_ps, ones_row[:, 0:128], bv_row[:, g0 * H:g1 * H],
                    start=False, stop=True)
                nc.vector.tensor_copy(
                    v_sb[:, tt, :, 0:H],
                    v_ps.rearrange("p (e h) -> p e h", e=gsz),
                )

        def emit_qt(p, sink):
            qt_ps = psp.tile([128, SL], F32, name="qt_ps", tag="sc")
            for c in range(DC):
                _mm(nc, qt_ps, wqk_sb[:, p, c, :, 0:128], xpk[:, c, :, 0:SL],
                    start=(c == 0), stop=False, perf_mode=DR)
            _mm(nc, qt_ps, bqk_sb[:, p, 0:128], ones_row[:, 0:SL],
                start=False, stop=True)
            q_sb = qkp.tile([128, SL], MM_DT, name="q_sb", tag="q")
            nc.vector.tensor_copy(q_sb, qt_ps)
            sink[0] = q_sb

        def emit_kt(p, ch, sink):
            sl = slice(ch * 512, (ch + 1) * 512)
            kt_ps = psp.tile([128, 512], F32, name=f"kt_ps{ch}", tag="sc")
            for c in range(DC):
                _mm(nc, kt_ps, wqk_sb[:, p, c, :, 128:256], xpk[:, c, :, sl],
                    start=(c == 0), stop=False, perf_mode=DR)
            _mm(nc, kt_ps, bqk_sb[:, p, 128:256], ones_row[:, 0:512],
                start=False, stop=True)
            k_sb = qkp.tile([128, 512], MM_DT, name=f"k_sb{ch}", tag=f"k{ch}")
            nc.vector.tensor_copy(k_sb, kt_ps)
            sink[1][ch] = k_sb

        def emit_proj(p):
            sink = [None, [None, None]]
            emit_qt(p, sink)
            emit_kt(p, 0, sink)
            emit_kt(p, 1, sink)
            return sink

        def emit_att(p, sink, fillers=()):
            q_sb, k_chunks = sink[0], sink[1]
            gi = GROUP_OF_PAIR[p]
            g0 = GROUPS[gi][0]
            v_sb = v_tiles[gi]
            g0_tiles = []
            for sub in range(2):
                g_row0 = smp.tile([1, SL], F32, name="g_row0")
                nc.sync.dma_start(
                    out=g_row0, in_=gates_sb[2 * p + sub:2 * p + sub + 1, :])
                g0_tiles.append(g_row0)

            eo_tiles = [
                pep.tile([H + 1, SL], F32, name=f"eo_ps{s}", tag="eo")
                for s in range(2)
            ]

            def emit_eo(ats, c):
                for sub in range(2):
                    i = 2 * p + sub - g0
                    _mm(nc, eo_tiles[sub],
                        v_sb[:, 2 * c:2 * c + 2, i, 0:H + 1], ats[sub],
                        start=(c == 0), stop=(c == KT // 2 - 1),
                        perf_mode=DR)

            # scores: alternate sub-expert row groups (rows 0-63 / 64-127)
            # on consecutive matmuls so disjoint row tiles overlap in the PE
            pending = []
            for c in range(KT // 2):
                scs = [psp.tile([128, 2, SL], F32, name=f"sc2_{s}", tag="sc")
                       for s in range(2)]
                for j in range(2):
                    kt = 2 * c + j
                    ksl = slice((kt % 4) * 128, (kt % 4 + 1) * 128)
                    for sub in range(2):
                        po = sub * 64
                        _mm(nc, scs[sub][:, j, :],
                            k_chunks[kt // 4][po:po + 64, ksl],
                            q_sb[po:po + 64, :], start=True, stop=True)
                ats = []
                for sub in range(2):
                    at2 = atp.tile([128, 2, SL], FP8, name="at2", tag="at")
                    nc.scalar.activation(at2, scs[sub], AF.Exp, scale=SCALE)
                    ats.append(at2)
                pending.append((ats, c))
                if len(pending) > 1:
                    emit_eo(*pending.pop(0))
                if c < len(fillers):
                    fillers[c]()
            emit_eo(*pending.pop(0))

            # gate/sumexp normalization of eoT -> combT[p]
            for sub in range(2):
                eo_ps = eo_tiles[sub]
                s_inv = smp.tile([1, SL], F32, name="s_inv")
                nc.vector.reciprocal(s_inv, eo_ps[H:H + 1, :])
                g_row = smp.tile([1, SL], F32, name="g_row")
                nc.vector.tensor_mul(g_row, s_inv, g0_tiles[sub])
                sc64 = smp.tile([H, SL], F32, name="sc64")
                nc.gpsimd.partition_broadcast(sc64, g_row)
                nc.vector.tensor_mul(
                    combT[p][sub * 64:sub * 64 + 64, :], eo_ps[0:H, :], sc64
                )

        # ---- pipelined schedule: PROJ(p+1) lands before ATT(p) so its
        # q/k copies hide under the previous pair's attention ----
        emit_vloop(0)
        finish_gates()
        proj = {0: emit_proj(0)}
        for p in range(EP):
            nxt = p + 1
            if nxt in PAIR_STARTS_GROUP:
                emit_vloop(PAIR_STARTS_GROUP[nxt])
            if nxt < EP:
                proj[nxt] = emit_proj(nxt)
            emit_att(p, proj.pop(p))

        # ---- out projection: for each token tile, one N=1024 chunk and one
        # N=256 chunk, sharing each combT[ht] stationary back-to-back ----
        OCH = [(0, 512), (512, 1024), (1024, 1280)]
        last_tt = SL // 128 - 1
        for tt in range(SL // 128):
            tsl = slice(tt * 128, (tt + 1) * 128)
            o_sb = io.tile([128, D], F32, name="o_sb", tag="o_sb")
            for ci, (c0, c1) in enumerate(OCH):
                op_ps = psp.tile([128, c1 - c0], F32, name=f"op{ci}", tag="sc")
                for ht in range(HT):
                    _mm(nc, op_ps, combT[ht][:, tsl], ow_sb[:, ht, c0:c1],
                        start=(ht == 0), stop=False)
                _mm(nc, op_ps, ones_row[:, 0:128], ob_sb[:, c0:c1],
                    start=False, stop=True)
                nc.vector.tensor_copy(o_sb[:, c0:c1], op_ps)
                if tt == last_tt:
                    # per-chunk stores so the kernel tail is only the last
                    # 256-wide chunk's copy + DMA
                    nc.sync.dma_start(out=out_d[tsl, c0:c1],
                                      in_=o_sb[:, c0:c1])
            if tt != last_tt:
                nc.sync.dma_start(out=out_d[tsl, :], in_=o_sb)


def declare_tensors(nc):
    xpk_d = nc.dram_tensor("xpk", [128, DC, 2, S], FP8, kind="ExternalInput").ap()
    wqk_d = nc.dram_tensor("wqk", [128, EP, DC, 2, 256], FP8,
                           kind="ExternalInput").ap()
    bqk_d = nc.dram_tensor("bqk", [EP, 256], MM_DT, kind="ExternalInput").ap()
    wv_d = nc.dram_tensor("wv", [128, DC, 2, E * H], FP8,
                          kind="ExternalInput").ap()
    bv_d = nc.dram_tensor("bv", [1, E * H], MM_DT, kind="ExternalInput").ap()
    rw_d = nc.dram_tensor("router_w", [128, DC, 2, 64], FP8,
                          kind="ExternalInput").ap()
    rb_d = nc.dram_tensor("router_b", [64], MM_DT, kind="ExternalInput").ap()
    ow_d = nc.dram_tensor("out_w", [D, D], MM_DT, kind="ExternalInput").ap()
    ob_d = nc.dram_tensor("out_b", [D], MM_DT, kind="ExternalInput").ap()
    out_d = nc.dram_tensor("out", [SL, D], F32, kind="ExternalOutput").ap()
    return (xpk_d, wqk_d, bqk_d, wv_d, bv_d, rw_d, rb_d, ow_d, ob_d, out_d)


def build_nc():
    nc = bacc.Bacc("TRN2", target_bir_lowering=False, debug=False,
                   num_devices=NCORES)
    tensors = declare_tensors(nc)
    with tile.TileContext(nc) as tc:
        _emit(tc, *tensors)
    nc.compile()
    return nc


_NC = None


def _get_nc():
    global _NC
    if _NC is None:
        _NC = build_nc()
    return _NC


def _pack_dr(a):
    """[D, N] f32 -> [128, DC, 2, N] fp8e4m3 (DoubleRow contraction layout)."""
    a = np.asarray(a, np.float32)
    n = a.shape[1]
    return np.ascontiguousarray(
        a.reshape(DC, 2, 128, n).transpose(2, 0, 1, 3).astype(NP_F8))


def make_in_maps(x, wqkv, bqkv, router_w, router_b, out_w, out_b):
    x = np.ascontiguousarray(np.asarray(x, np.float32))
    wqkv = np.asarray(wqkv, np.float32)
    bqkv = np.asarray(bqkv, np.float32)
    wq = wqkv[:, :, 0:H].reshape(EP, 2, D, H).transpose(0, 2, 1, 3).reshape(EP, D, 128)
    wk = wqkv[:, :, H:2 * H].reshape(EP, 2, D, H).transpose(0, 2, 1, 3).reshape(EP, D, 128)
    wqk = np.concatenate([wq, wk], axis=-1)  # [EP, D, 256]
    # -> [128, EP, DC, 2, 256] fp8
    wqk_pk = np.ascontiguousarray(
        wqk.reshape(EP, DC, 2, 128, 256).transpose(3, 0, 1, 2, 4).astype(NP_F8))
    bq = bqkv[:, 0:H].reshape(EP, 128)
    bk = bqkv[:, H:2 * H].reshape(EP, 128)
    bqk = np.concatenate([bq, bk], axis=-1)
    wv = wqkv[:, :, 2 * H:3 * H].transpose(1, 0, 2).reshape(D, E * H)
    bv = bqkv[:, 2 * H:3 * H].reshape(1, E * H)

    def _c(a):
        return np.ascontiguousarray(np.asarray(a, np.float32).astype(NP_MM))
    shared = {
        "wqk": wqk_pk, "bqk": _c(bqk), "wv": _pack_dr(wv), "bv": _c(bv),
        "router_w": _pack_dr(np.pad(np.asarray(router_w, np.float32),
                                    ((0, 0), (0, 64 - E)))),
        "router_b": _c(np.pad(np.asarray(router_b, np.float32), (0, 64 - E))),
        "out_w": _c(out_w), "out_b": _c(out_b),
    }
    in_maps = []
    for c in range(NCORES):
        b, half = c // 2, c % 2
        xb = x[b]
        if half == 0:
            x_ctx = xb
        else:
            x_ctx = np.concatenate([xb[SL:], xb[:SL]], axis=0)
        in_maps.append({"xpk": _pack_dr(x_ctx.T), **shared})
    return in_maps


def gather_out(results):
    out = np.empty((B, S, D), np.float32)
    for c in range(NCORES):
        b, half = c // 2, c % 2
        out[b, half * SL:(half + 1) * SL] = results[c]["out"]
    return out


def kernel(x, wqkv, bqkv, router_w, router_b, out_w, out_b):
    nc = _get_nc()
    in_maps = make_in_maps(x, wqkv, bqkv, router_w, router_b, out_w, out_b)
    res = run_bass_kernel_spmd(nc, in_maps, core_ids=list(range(NCORES)))
    return gather_out(res.results)


# revision 11
# speedup vs baseline: 2.4620x; 1.9204x over previous
"""MoE-attention kernel for 8 Trainium2 NeuronCores.

Sharding: token-parallel. Core c handles sequence b = c//2, query-token half
half = c%2 (512 query tokens). Each core computes all 20 experts for its
query tokens; K/V context is the full 1024-token sequence, fed with the
local half FIRST (attention is permutation-invariant over key positions).
No collectives: out_proj partial sums are avoided by giving every core the
full feature dim for its own query tokens.

v4 layout strategy (fp8 DoubleRow matmuls for projections + attn*v):
  x, wv, wqk, rw  host-packed fp8e4m3 as [128, 5, 2, N]: contraction d-dim
                  split into 5 chunks of 256 = (2 planes x 128 partitions),
                  consumed by DoubleRow matmuls (measured ~1.9x bf16 FLOP/s)
  biases          bf16, added via regular K=1 matmuls into the same PSUM group
  q/k             PSUM f32 -> SBUF bf16 (scores stay bf16: contraction is only
                  64, served by PE quadrant pairs at full rate; fp8 DR cannot
                  beat that shape)
  scoresT [128kt, 2, 512qt]  2 key-tiles per PSUM tile -> one big exp
  attnT = exp(scoresT*scale) emitted as fp8 (no max-subtraction: scores are
                  bounded, exp < ~50 fits fp8e4m3 range 448)
  v       [128t, kt, e, 65] fp8 with ones column -> sumexp row
  eoT     [65h, 512qt] = DoubleRow over (v 2-key-tile planes, attn planes)
  combT   10 per-ht tiles [128h', 512qt] = eoT * gate/sumexp   (bf16)
  out     [512t, 1280] = combT.T @ out_w + out_b, all bf16 (fp8 out_proj
                  fails the accuracy budget; measured numerically)
  software pipeline: PROJ(p+1) emitted before ATT(p), so the projection
  copies always hide under the previous pair's attention; within ATT,
  exp(unit u) on Act overlaps eo-matmuls(u-1) on PE
"""

import numpy as np

import concourse.bass as bass
import concourse.mybir as mybir
import concourse.tile as tile
from concourse import bacc
from concourse.bass_utils import run_bass_kernel_spmd

F32 = mybir.dt.float32
BF16 = mybir.dt.bfloat16
FP8 = mybir.dt.float8e4
DR = mybir.MatmulPerfMode.DoubleRow
MM_DT = BF16
try:
    import ml_dtypes as _mld
    NP_MM = np.dtype(_mld.bfloat16)
    NP_F8 = np.dtype(_mld.float8_e4m3)
except Exception as e:  # pragma: no cover
    raise RuntimeError("ml_dtypes required for bf16/fp8 host packing") from e
AF = mybir.ActivationFunctionType

B = 4
S = 1024          # sequence length (full context per core)
D = 1280          # d_model
E = 20            # experts
EP = E // 2       # expert pairs
H = 64            # head dim
SL = 512          # local query tokens per core
DC = 5            # d-model DoubleRow chunks (5 x (2 x 128) = 1280)
DT = D // 128     # 10 d-tiles (out_proj contraction)
HT = D // 128     # 10 h'-tiles
KT = S // 128     # 8 key tiles
SCALE = float(H) ** -0.5
NCORES = 8
GROUPS = [(0, 8), (8, 16), (16, 20)]
GROUP_OF_PAIR = {p: gi for gi, (g0, g1) in enumerate(GROUPS)
                 for p in range(g0 // 2, g1 // 2)}
PAIR_STARTS_GROUP = {g0 // 2: gi for gi, (g0, g1) in enumerate(GROUPS)}


def _mm(nc, out, lhsT, rhs, **kw):
    nc.tensor.matmul(out, lhsT, rhs, **kw)


def _emit(tc, xpk_d, wqk_d, bqk_d, wv_d, bv_d, rw_d, rb_d, ow_d, ob_d, out_d):
    nc = tc.nc
    with (
        tc.tile_pool(name="const", bufs=1) as const,
        tc.tile_pool(name="io", bufs=2) as io,
        tc.tile_pool(name="vp", bufs=2) as vp,
        tc.tile_pool(name="qk", bufs=2) as qkp,
        tc.tile_pool(name="at", bufs=6) as atp,
        tc.tile_pool(name="sm", bufs=2) as smp,
        tc.tile_pool(name="scp", bufs=2, space="PSUM") as scp,
        tc.tile_pool(name="ppp", bufs=2, space="PSUM") as ppp,
        tc.tile_pool(name="pe", bufs=2, space="PSUM") as pep,
    ):
        # ---- constants + all weights up-front (few large DMAs) ----
        ones_f32 = const.tile([128, 512], F32, name="ones_f32")
        nc.gpsimd.memset(ones_f32, 1.0)
        ones_row = const.tile([1, S], MM_DT, name="ones_row")
        nc.vector.tensor_copy(ones_row[:, 0:512], ones_f32[0:1, :])
        nc.vector.tensor_copy(ones_row[:, 512:1024], ones_f32[0:1, :])
        ones20 = const.tile([E, 1], MM_DT, name="ones20")
        nc.vector.tensor_copy(ones20, ones_f32[0:E, 0:1])

        # router weights first (tiny), then x packed as three c-chunks so the
        # router chain starts as soon as the first chunk lands
        # router weights padded to 64 output cols: DoubleRow with M=20
        # (tile col 32) fails the walrus ISA check; M=64 passes
        rw_sb = const.tile([128, DC, 2, 64], FP8, name="rw_sb")
        nc.sync.dma_start(out=rw_sb, in_=rw_d)
        rb_sb = const.tile([1, 64], MM_DT, name="rb_sb")
        nc.sync.dma_start(out=rb_sb, in_=rb_d[None, :])

        # DMA issue queues: sync (SP) for early/critical transfers, gpsimd
        # (Pool, ~free dispatch) for the bulk tail. Act issues none: its
        # queue is the exp bottleneck.
        xpk = const.tile([128, DC, 2, S], FP8, name="xpk")
        nc.sync.dma_start(out=xpk[:, 0:1], in_=xpk_d[:, 0:1])
        nc.gpsimd.dma_start(out=xpk[:, 1:3], in_=xpk_d[:, 1:3])
        nc.sync.dma_start(out=xpk[:, 3:5], in_=xpk_d[:, 3:5])

        wv_sb = const.tile([128, DC, 2, E * H], FP8, name="wv_sb")
        nc.gpsimd.dma_start(out=wv_sb[:, :, :, 0:8 * H],
                            in_=wv_d[:, :, :, 0:8 * H])
        nc.sync.dma_start(out=wv_sb[:, :, :, 8 * H:], in_=wv_d[:, :, :, 8 * H:])
        bv_row = const.tile([1, E * H], MM_DT, name="bv_row")
        nc.sync.dma_start(out=bv_row, in_=bv_d)

        wqk_sb = const.tile([128, EP, DC, 2, 256], FP8, name="wqk_sb")
        nc.sync.dma_start(out=wqk_sb[:, 0:2], in_=wqk_d[:, 0:2])
        nc.gpsimd.dma_start(out=wqk_sb[:, 2:6], in_=wqk_d[:, 2:6])
        nc.gpsimd.dma_start(out=wqk_sb[:, 6:], in_=wqk_d[:, 6:])
        bqk_cols = const.tile([128, 2, EP], F32, name="bqk_cols")
        nc.sync.dma_start(out=bqk_cols, in_=bqk_d)

        ow_sb = const.tile([128, DT, D], MM_DT, name="ow_sb")
        ow_in = ow_d.rearrange("(t p) n -> p t n", p=128)
        nc.gpsimd.dma_start(out=ow_sb[:, 0:DT // 2], in_=ow_in[:, 0:DT // 2])
        nc.gpsimd.dma_start(out=ow_sb[:, DT // 2:], in_=ow_in[:, DT // 2:])
        ob_sb = const.tile([1, D], MM_DT, name="ob_sb")
        nc.sync.dma_start(out=ob_sb, in_=ob_d[None, :])

        # per-ht combined-feature tiles (separate so out_proj's ht chain only
        # waits on the normalize that wrote that ht)
        combT = [const.tile([128, SL], MM_DT, name=f"combT{h}")
                 for h in range(HT)]

        # ---- router logits (gates finished after group-0 V so the PE
        # never waits on the Act queue's DMA issues) ----
        exp_router = const.tile([E, SL], MM_DT, name="exp_router")
        gates_sb = const.tile([E, SL], F32, name="gates_sb")
        inv_rsum = const.tile([1, SL], F32, name="inv_rsum")
        inv_rep = const.tile([E, SL], F32, name="inv_rep")

        rt_ps = ppp.tile([64, SL], F32, name="rt_ps", tag="pp")
        for c in range(DC):
            _mm(nc, rt_ps, rw_sb[:, c], xpk[:, c, :, 0:SL],
                start=(c == 0), stop=False, perf_mode=DR)
        _mm(nc, rt_ps, rb_sb, ones_row[:, 0:SL], start=False, stop=True)
        nc.scalar.activation(exp_router, rt_ps[0:E, :], AF.Exp)

        g_all = const.tile([1, E, SL], F32, name="g_all")

        def finish_gates():
            rs_ps = ppp.tile([1, SL], F32, name="rs_ps", tag="pp")
            _mm(nc, rs_ps, ones20, exp_router, start=True, stop=True)
            nc.vector.reciprocal(inv_rsum, rs_ps)
            nc.gpsimd.partition_broadcast(inv_rep, inv_rsum)
            nc.vector.tensor_mul(gates_sb, exp_router, inv_rep)
            # all 20 gate rows onto partition 0 in ONE transfer; per-pair
            # normalizes then just slice g_all (the per-pair row DMAs used
            # to arrive late and stall the eo-PSUM handoff)
            nc.sync.dma_start(out=g_all, in_=gates_sb[:, :])

        v_tiles = {}

        def alloc_vgroup(gi):
            g0, g1 = GROUPS[gi]
            gsz = g1 - g0
            # row width padded 65 -> 68: dual-fp8 Ldweights requires the
            # plane (kt) byte-stride to be a multiple of 16 (gsz*68 = 544/272)
            v_sb = vp.tile([128, KT, gsz, 68], FP8, name="v_sb", tag="vg")
            v_tiles[gi] = v_sb
            nc.vector.tensor_copy(
                v_sb[:, :, :, H],
                ones_f32[:, 0:KT * gsz].rearrange("p (a b) -> p a b", a=KT),
            )

        def emit_vchain(gi, tt):
            g0, g1 = GROUPS[gi]
            gsz = g1 - g0
            v_sb = v_tiles[gi]
            v_ps = ppp.tile([128, gsz * H], F32, name="v_ps", tag="pp")
            for c in range(DC):
                _mm(nc, v_ps, xpk[:, c, :, tt * 128:(tt + 1) * 128],
                    wv_sb[:, c, :, g0 * H:g1 * H],
                    start=(c == 0), stop=False, perf_mode=DR)
            _mm(nc, v_ps, ones_row[:, 0:128], bv_row[:, g0 * H:g1 * H],
                start=False, stop=True)
            nc.vector.tensor_copy(
                v_sb[:, tt, :, 0:H],
                v_ps.rearrange("p (e h) -> p e h", e=gsz),
            )

        def emit_qt(p, sink):
            qt_ps = ppp.tile([128, SL], F32, name="qt_ps", tag="pp")
            for c in range(DC):
                _mm(nc, qt_ps, wqk_sb[:, p, c, :, 0:128], xpk[:, c, :, 0:SL],
                    start=(c == 0), stop=(c == DC - 1), perf_mode=DR)
            q_sb = qkp.tile([128, SL], MM_DT, name="q_sb", tag="q")
            nc.vector.tensor_scalar_add(q_sb, qt_ps, bqk_cols[:, 0, p:p + 1])
            sink[0] = q_sb

        def emit_kt(p, ch, sink):
            sl = slice(ch * 512, (ch + 1) * 512)
            kt_ps = ppp.tile([128, 512], F32, name=f"kt_ps{ch}", tag="pp")
            for c in range(DC):
                _mm(nc, kt_ps, wqk_sb[:, p, c, :, 128:256], xpk[:, c, :, sl],
                    start=(c == 0), stop=(c == DC - 1), perf_mode=DR)
            k_sb = qkp.tile([128, 512], MM_DT, name=f"k_sb{ch}", tag=f"k{ch}")
            nc.vector.tensor_scalar_add(k_sb, kt_ps, bqk_cols[:, 1, p:p + 1])
            sink[1][ch] = k_sb

        def emit_proj(p):
            sink = [None, [None, None]]
            emit_qt(p, sink)
            emit_kt(p, 0, sink)
            emit_kt(p, 1, sink)
            return sink

        def emit_att(p, sink, fillers=()):
            q_sb, k_chunks = sink[0], sink[1]
            gi = GROUP_OF_PAIR[p]
            g0 = GROUPS[gi][0]
            v_sb = v_tiles[gi]
            g0_tiles = [g_all[:, 2 * p + sub, :] for sub in range(2)]

            eo_tiles = [
                pep.tile([H + 1, SL], F32, name=f"eo_ps{s}", tag="eo")
                for s in range(2)
            ]

            def emit_eo(ats, c):
                for sub in range(2):
                    i = 2 * p + sub - g0
                    _mm(nc, eo_tiles[sub],
                        v_sb[:, 2 * c:2 * c + 2, i, 0:H + 1], ats[sub],
                        start=(c == 0), stop=(c == KT // 2 - 1),
                        perf_mode=DR)

            # scores: alternate sub-expert row groups (rows 0-63 / 64-127)
            # on consecutive matmuls so disjoint row tiles overlap in the PE;
            # eo trails the exp stream by two c-steps (6 at2 buffers) so the
            # PE never waits on the current exp
            pending = []
            for c in range(KT // 2):
                scs = [scp.tile([128, 2, SL], F32, name=f"sc2_{s}", tag="sc")
                       for s in range(2)]
                for j in range(2):
                    kt = 2 * c + j
                    ksl = slice((kt % 4) * 128, (kt % 4 + 1) * 128)
                    for sub in range(2):
                        po = sub * 64
                        _mm(nc, scs[sub][:, j, :],
                            k_chunks[kt // 4][po:po + 64, ksl],
                            q_sb[po:po + 64, :], start=True, stop=True)
                ats = []
                for sub in range(2):
                    at2 = atp.tile([128, 2, SL], FP8, name="at2", tag="at")
                    nc.scalar.activation(at2, scs[sub], AF.Exp, scale=SCALE)
                    ats.append(at2)
                pending.append((ats, c))
                if len(pending) > 2:
                    emit_eo(*pending.pop(0))
                if c < len(fillers):
                    fillers[c]()
            while pending:
                emit_eo(*pending.pop(0))

            # gate/sumexp normalization of eoT -> combT[p]
            for sub in range(2):
                eo_ps = eo_tiles[sub]
                s_inv = smp.tile([1, SL], F32, name="s_inv")
                nc.vector.reciprocal(s_inv, eo_ps[H:H + 1, :])
                g_row = smp.tile([1, SL], F32, name="g_row")
                nc.vector.tensor_mul(g_row, s_inv, g0_tiles[sub])
                sc64 = smp.tile([H, SL], F32, name="sc64")
                nc.gpsimd.partition_broadcast(sc64, g_row)
                nc.vector.tensor_mul(
                    combT[p][sub * 64:sub * 64 + 64, :], eo_ps[0:H, :], sc64
                )

        # ---- pipelined schedule: PROJ(p+1) lands before ATT(p) so its
        # q/k copies hide under the previous pair's attention; the NEXT
        # group's 8 V-chains are spread as fillers (2 per pair) across the
        # current group's 4 pairs, instead of a serial 15us PE block at the
        # group boundary that starves Act/DVE ----
        alloc_vgroup(0)
        for tt in range(KT):
            emit_vchain(0, tt)
        finish_gates()
        proj = {0: emit_proj(0)}
        for p in range(EP):
            nxt = p + 1
            gi = GROUP_OF_PAIR[p]
            fillers = ()
            if gi + 1 < len(GROUPS):
                idx = p - GROUPS[gi][0] // 2
                if idx == 0:
                    alloc_vgroup(gi + 1)
                fillers = tuple(
                    (lambda g=gi + 1, t=t: emit_vchain(g, t))
                    for t in (2 * idx, 2 * idx + 1))
            if nxt < EP:
                proj[nxt] = emit_proj(nxt)
            emit_att(p, proj.pop(p), fillers)

        # ---- out projection: for each token tile, one N=1024 chunk and one
        # N=256 chunk, sharing each combT[ht] stationary back-to-back ----
        OCH = [(0, 512), (512, 1024), (1024, 1280)]
        last_tt = SL // 128 - 1
        for tt in range(SL // 128):
            tsl = slice(tt * 128, (tt + 1) * 128)
            o_sb = io.tile([128, D], F32, name="o_sb", tag="o_sb")
            for ci, (c0, c1) in enumerate(OCH):
                op_ps = ppp.tile([128, c1 - c0], F32, name=f"op{ci}", tag="pp")
                for ht in range(HT):
                    _mm(nc, op_ps, combT[ht][:, tsl], ow_sb[:, ht, c0:c1],
                        start=(ht == 0), stop=False)
                _mm(nc, op_ps, ones_row[:, 0:128], ob_sb[:, c0:c1],
                    start=False, stop=True)
                nc.vector.tensor_copy(o_sb[:, c0:c1], op_ps)
                if tt == last_tt:
                    # per-chunk stores so the kernel tail is only the last
                    # 256-wide chunk's copy + DMA
                    nc.sync.dma_start(out=out_d[tsl, c0:c1],
                                      in_=o_sb[:, c0:c1])
            if tt != last_tt:
                nc.sync.dma_start(out=out_d[tsl, :], in_=o_sb)


def declare_tensors(nc):
    xpk_d = nc.dram_tensor("xpk", [128, DC, 2, S], FP8, kind="ExternalInput").ap()
    wqk_d = nc.dram_tensor("wqk", [128, EP, DC, 2, 256], FP8,
                           kind="ExternalInput").ap()
    bqk_d = nc.dram_tensor("bqk", [128, 2, EP], F32, kind="ExternalInput").ap()
    wv_d = nc.dram_tensor("wv", [128, DC, 2, E * H], FP8,
                          kind="ExternalInput").ap()
    bv_d = nc.dram_tensor("bv", [1, E * H], MM_DT, kind="ExternalInput").ap()
    rw_d = nc.dram_tensor("router_w", [128, DC, 2, 64], FP8,
                          kind="ExternalInput").ap()
    rb_d = nc.dram_tensor("router_b", [64], MM_DT, kind="ExternalInput").ap()
    ow_d = nc.dram_tensor("out_w", [D, D], MM_DT, kind="ExternalInput").ap()
    ob_d = nc.dram_tensor("out_b", [D], MM_DT, kind="ExternalInput").ap()
    out_d = nc.dram_tensor("out", [SL, D], F32, kind="ExternalOutput").ap()
    return (xpk_d, wqk_d, bqk_d, wv_d, bv_d, rw_d, rb_d, ow_d, ob_d, out_d)


def build_nc():
    nc = bacc.Bacc("TRN2", target_bir_lowering=False, debug=False,
                   num_devices=NCORES)
    tensors = declare_tensors(nc)
    with tile.TileContext(nc) as tc:
        _emit(tc, *tensors)
    nc.compile()
    return nc


_NC = None


def _get_nc():
    global _NC
    if _NC is None:
        _NC = build_nc()
    return _NC


def _pack_dr(a):
    """[D, N] f32 -> [128, DC, 2, N] fp8e4m3 (DoubleRow contraction layout)."""
    a = np.asarray(a, np.float32)
    n = a.shape[1]
    return np.ascontiguousarray(
        a.reshape(DC, 2, 128, n).transpose(2, 0, 1, 3).astype(NP_F8))


def make_in_maps(x, wqkv, bqkv, router_w, router_b, out_w, out_b):
    x = np.ascontiguousarray(np.asarray(x, np.float32))
    wqkv = np.asarray(wqkv, np.float32)
    bqkv = np.asarray(bqkv, np.float32)
    wq = wqkv[:, :, 0:H].reshape(EP, 2, D, H).transpose(0, 2, 1, 3).reshape(EP, D, 128)
    wk = wqkv[:, :, H:2 * H].reshape(EP, 2, D, H).transpose(0, 2, 1, 3).reshape(EP, D, 128)
    wqk = np.concatenate([wq, wk], axis=-1)  # [EP, D, 256]
    # -> [128, EP, DC, 2, 256] fp8
    wqk_pk = np.ascontiguousarray(
        wqk.reshape(EP, DC, 2, 128, 256).transpose(3, 0, 1, 2, 4).astype(NP_F8))
    bq = bqkv[:, 0:H].reshape(EP, 128)     # row layout (e0h0-63, e1h0-63)
    bk = bqkv[:, H:2 * H].reshape(EP, 128)
    # f32 per-partition bias columns for tensor_scalar_add: [128, {q,k}, EP]
    bqk_cols = np.ascontiguousarray(
        np.stack([bq.T, bk.T], axis=1).astype(np.float32))
    wv = wqkv[:, :, 2 * H:3 * H].transpose(1, 0, 2).reshape(D, E * H)
    bv = bqkv[:, 2 * H:3 * H].reshape(1, E * H)

    def _c(a):
        return np.ascontiguousarray(np.asarray(a, np.float32).astype(NP_MM))
    shared = {
        "wqk": wqk_pk, "bqk": bqk_cols, "wv": _pack_dr(wv), "bv": _c(bv),
        "router_w": _pack_dr(np.pad(np.asarray(router_w, np.float32),
                                    ((0, 0), (0, 64 - E)))),
        "router_b": _c(np.pad(np.asarray(router_b, np.float32), (0, 64 - E))),
        "out_w": _c(out_w), "out_b": _c(out_b),
    }
    in_maps = []
    for c in range(NCORES):
        b, half = c // 2, c % 2
        xb = x[b]
        if half == 0:
            x_ctx = xb
        else:
            x_ctx = np.concatenate([xb[SL:], xb[:SL]], axis=0)
        in_maps.append({"xpk": _pack_dr(x_ctx.T), **shared})
    return in_maps


def gather_out(results):
    out = np.empty((B, S, D), np.float32)
    for c in range(NCORES):
        b, half = c // 2, c % 2
        out[b, half * SL:(half + 1) * SL] = results[c]["out"]
    return out


def kernel(x, wqkv, bqkv, router_w, router_b, out_w, out_b):
    nc = _get_nc()
    in_maps = make_in_maps(x, wqkv, bqkv, router_w, router_b, out_w, out_b)
    res = run_bass_kernel_spmd(nc, in_maps, core_ids=list(range(NCORES)))
    return gather_out(res.results)


# revision 19
# speedup vs baseline: 2.5851x; 1.0500x over previous
"""MoE-attention kernel for 8 Trainium2 NeuronCores.

Sharding: token-parallel. Core c handles sequence b = c//2, query-token half
half = c%2 (512 query tokens). Each core computes all 20 experts for its
query tokens; K/V context is the full 1024-token sequence, fed with the
local half FIRST (attention is permutation-invariant over key positions).
No collectives: out_proj partial sums are avoided by giving every core the
full feature dim for its own query tokens.

v4 layout strategy (fp8 DoubleRow matmuls for projections + attn*v):
  x, wv, wqk, rw  host-packed fp8e4m3 as [128, 5, 2, N]: contraction d-dim
                  split into 5 chunks of 256 = (2 planes x 128 partitions),
                  consumed by DoubleRow matmuls (measured ~1.9x bf16 FLOP/s)
  biases          bf16, added via regular K=1 matmuls into the same PSUM group
  q/k             PSUM f32 -> SBUF bf16 (scores stay bf16: contraction is only
                  64, served by PE quadrant pairs at full rate; fp8 DR cannot
                  beat that shape)
  scoresT [128kt, 2, 512qt]  2 key-tiles per PSUM tile -> one big exp
  attnT = exp(scoresT*scale) emitted as fp8 (no max-subtraction: scores are
                  bounded, exp < ~50 fits fp8e4m3 range 448)
  v       [128t, kt, e, 65] fp8 with ones column -> sumexp row
  eoT     [65h, 512qt] = DoubleRow over (v 2-key-tile planes, attn planes)
  combT   10 per-ht tiles [128h', 512qt] = eoT * gate/sumexp   (bf16)
  out     [512t, 1280] = combT.T @ out_w + out_b, all bf16 (fp8 out_proj
                  fails the accuracy budget; measured numerically)
  software pipeline: PROJ(p+1) emitted before ATT(p), so the projection
  copies always hide under the previous pair's attention; within ATT,
  exp(unit u) on Act overlaps eo-matmuls(u-1) on PE
"""

import numpy as np

import concourse.bass as bass
import concourse.mybir as mybir
import concourse.tile as tile
from concourse import bacc
from concourse.bass_utils import run_bass_kernel_spmd

F32 = mybir.dt.float32
BF16 = mybir.dt.bfloat16
FP8 = mybir.dt.float8e4
DR = mybir.MatmulPerfMode.DoubleRow
MM_DT = BF16
try:
    import ml_dtypes as _mld
    NP_MM = np.dtype(_mld.bfloat16)
    NP_F8 = np.dtype(_mld.float8_e4m3)
except Exception as e:  # pragma: no cover
    raise RuntimeError("ml_dtypes required for bf16/fp8 host packing") from e
AF = mybir.ActivationFunctionType

B = 4
S = 1024          # sequence length (full context per core)
D = 1280          # d_model
E = 20            # experts
EP = E // 2       # expert pairs
H = 64            # head dim
SL = 512          # local query tokens per core
DC = 5            # d-model DoubleRow chunks (5 x (2 x 128) = 1280)
DT = D // 128     # 10 d-tiles (out_proj contraction)
HT = D // 128     # 10 h'-tiles
KT = S // 128     # 8 key tiles
SCALE = float(H) ** -0.5
NCORES = 8
GROUPS = [(0, 8), (8, 16), (16, 20)]
GROUP_OF_PAIR = {p: gi for gi, (g0, g1) in enumerate(GROUPS)
                 for p in range(g0 // 2, g1 // 2)}
PAIR_STARTS_GROUP = {g0 // 2: gi for gi, (g0, g1) in enumerate(GROUPS)}


def _mm(nc, out, lhsT, rhs, **kw):
    nc.tensor.matmul(out, lhsT, rhs, **kw)


def _emit(tc, xpk_d, wqk_d, bqk_d, wv_d, bv_d, rw_d, rb_d, ow_d, ob_d, out_d):
    nc = tc.nc
    with (
        tc.tile_pool(name="const", bufs=1) as const,
        tc.tile_pool(name="io", bufs=2) as io,
        tc.tile_pool(name="vp", bufs=2) as vp,
        tc.tile_pool(name="qk", bufs=2) as qkp,
        tc.tile_pool(name="at", bufs=6) as atp,
        tc.tile_pool(name="sm", bufs=2) as smp,
        tc.tile_pool(name="scp", bufs=2, space="PSUM") as scp,
        tc.tile_pool(name="ppp", bufs=2, space="PSUM") as ppp,
        tc.tile_pool(name="pe", bufs=2, space="PSUM") as pep,
    ):
        # ---- constants + all weights up-front (few large DMAs) ----
        ones_f32 = const.tile([128, 512], F32, name="ones_f32")
        nc.gpsimd.memset(ones_f32, 1.0)
        ones_row = const.tile([1, S], MM_DT, name="ones_row")
        nc.vector.tensor_copy(ones_row[:, 0:512], ones_f32[0:1, :])
        nc.vector.tensor_copy(ones_row[:, 512:1024], ones_f32[0:1, :])
        ones20 = const.tile([E, 1], MM_DT, name="ones20")
        nc.vector.tensor_copy(ones20, ones_f32[0:E, 0:1])

        # router weights first (tiny), then x packed as three c-chunks so the
        # router chain starts as soon as the first chunk lands
        # router weights padded to 64 output cols: DoubleRow with M=20
        # (tile col 32) fails the walrus ISA check; M=64 passes
        rw_sb = const.tile([128, DC, 2, 64], FP8, name="rw_sb")
        nc.sync.dma_start(out=rw_sb, in_=rw_d)
        rb_sb = const.tile([1, 64], MM_DT, name="rb_sb")
        nc.sync.dma_start(out=rb_sb, in_=rb_d[None, :])

        # DMA issue queues: sync (SP) for early/critical transfers, gpsimd
        # (Pool, ~free dispatch) for the bulk tail. Act issues none: its
        # queue is the exp bottleneck.
        xpk = const.tile([128, DC, 2, S], FP8, name="xpk")
        nc.sync.dma_start(out=xpk[:, 0:1], in_=xpk_d[:, 0:1])
        nc.gpsimd.dma_start(out=xpk[:, 1:3], in_=xpk_d[:, 1:3])
        nc.sync.dma_start(out=xpk[:, 3:5], in_=xpk_d[:, 3:5])

        wv_sb = const.tile([128, DC, 2, E * H], FP8, name="wv_sb")
        nc.gpsimd.dma_start(out=wv_sb[:, :, :, 0:8 * H],
                            in_=wv_d[:, :, :, 0:8 * H])
        nc.sync.dma_start(out=wv_sb[:, :, :, 8 * H:], in_=wv_d[:, :, :, 8 * H:])
        bv_row = const.tile([1, E * H], MM_DT, name="bv_row")
        nc.sync.dma_start(out=bv_row, in_=bv_d)

        wqk_sb = const.tile([128, EP, DC, 2, 256], FP8, name="wqk_sb")
        nc.sync.dma_start(out=wqk_sb[:, 0:2], in_=wqk_d[:, 0:2])
        nc.gpsimd.dma_start(out=wqk_sb[:, 2:6], in_=wqk_d[:, 2:6])
        nc.gpsimd.dma_start(out=wqk_sb[:, 6:], in_=wqk_d[:, 6:])
        bqk_cols = const.tile([128, 2, EP], F32, name="bqk_cols")
        nc.sync.dma_start(out=bqk_cols, in_=bqk_d)

        ow_sb = const.tile([128, DT, D], MM_DT, name="ow_sb")
        ow_in = ow_d.rearrange("(t p) n -> p t n", p=128)
        nc.gpsimd.dma_start(out=ow_sb[:, 0:DT // 2], in_=ow_in[:, 0:DT // 2])
        nc.gpsimd.dma_start(out=ow_sb[:, DT // 2:], in_=ow_in[:, DT // 2:])
        ob_sb = const.tile([1, D], MM_DT, name="ob_sb")
        nc.sync.dma_start(out=ob_sb, in_=ob_d[None, :])

        # f32 broadcast bias planes: the V and out-proj bias matmuls (24+12
        # K=1 matmuls, ~310ns PE each) become part of the existing DVE
        # PSUM->SBUF copies via tensor_add against these
        bv32 = const.tile([1, E * H], F32, name="bv32")
        nc.vector.tensor_copy(bv32, bv_row)
        bv_bc = const.tile([128, E * H], F32, name="bv_bc")
        nc.gpsimd.partition_broadcast(bv_bc, bv32)
        ob32 = const.tile([1, D], F32, name="ob32")
        nc.vector.tensor_copy(ob32, ob_sb)
        ob_bc = const.tile([128, D], F32, name="ob_bc")
        nc.gpsimd.partition_broadcast(ob_bc, ob32)

        # per-ht combined-feature tiles (separate so out_proj's ht chain only
        # waits on the normalize that wrote that ht)
        combT = [const.tile([128, SL], MM_DT, name=f"combT{h}")
                 for h in range(HT)]

        # ---- router logits (gates finished after group-0 V so the PE
        # never waits on the Act queue's DMA issues) ----
        exp_router = const.tile([E, SL], MM_DT, name="exp_router")
        gates_sb = const.tile([E, SL], F32, name="gates_sb")
        inv_rsum = const.tile([1, SL], F32, name="inv_rsum")
        inv_rep = const.tile([E, SL], F32, name="inv_rep")

        rt_ps = ppp.tile([64, SL], F32, name="rt_ps", tag="pp")
        for c in range(DC):
            _mm(nc, rt_ps, rw_sb[:, c], xpk[:, c, :, 0:SL],
                start=(c == 0), stop=False, perf_mode=DR)
        _mm(nc, rt_ps, rb_sb, ones_row[:, 0:SL], start=False, stop=True)
        nc.scalar.activation(exp_router, rt_ps[0:E, :], AF.Exp)

        g_all = const.tile([1, E, SL], F32, name="g_all")

        def finish_gates():
            rs_ps = ppp.tile([1, SL], F32, name="rs_ps", tag="pp")
            _mm(nc, rs_ps, ones20, exp_router, start=True, stop=True)
            nc.vector.reciprocal(inv_rsum, rs_ps)
            nc.gpsimd.partition_broadcast(inv_rep, inv_rsum)
            nc.vector.tensor_mul(gates_sb, exp_router, inv_rep)
            # all 20 gate rows onto partition 0 in ONE transfer; per-pair
            # normalizes then just slice g_all (the per-pair row DMAs used
            # to arrive late and stall the eo-PSUM handoff)
            nc.sync.dma_start(out=g_all, in_=gates_sb[:, :])

        v_tiles = {}

        def alloc_vgroup(gi):
            g0, g1 = GROUPS[gi]
            gsz = g1 - g0
            # row width padded 65 -> 68: dual-fp8 Ldweights requires the
            # plane (kt) byte-stride to be a multiple of 16 (gsz*68 = 544/272)
            v_sb = vp.tile([128, KT, gsz, 68], FP8, name="v_sb", tag="vg")
            v_tiles[gi] = v_sb
            nc.vector.tensor_copy(
                v_sb[:, :, :, H],
                ones_f32[:, 0:KT * gsz].rearrange("p (a b) -> p a b", a=KT),
            )

        def emit_vchain(gi, tt):
            g0, g1 = GROUPS[gi]
            gsz = g1 - g0
            v_sb = v_tiles[gi]
            v_ps = ppp.tile([128, gsz * H], F32, name="v_ps", tag="pp")
            for c in range(DC):
                _mm(nc, v_ps, xpk[:, c, :, tt * 128:(tt + 1) * 128],
                    wv_sb[:, c, :, g0 * H:g1 * H],
                    start=(c == 0), stop=(c == DC - 1), perf_mode=DR)
            nc.vector.tensor_add(
                v_sb[:, tt, :, 0:H],
                v_ps.rearrange("p (e h) -> p e h", e=gsz),
                bv_bc[:, g0 * H:g1 * H].rearrange("p (e h) -> p e h", e=gsz),
            )

        def emit_qt(p, sink):
            qt_ps = ppp.tile([128, SL], F32, name="qt_ps", tag="pp")
            for c in range(DC):
                _mm(nc, qt_ps, wqk_sb[:, p, c, :, 0:128], xpk[:, c, :, 0:SL],
                    start=(c == 0), stop=(c == DC - 1), perf_mode=DR)
            q_sb = qkp.tile([128, SL], MM_DT, name="q_sb", tag="q")
            nc.vector.tensor_scalar_add(q_sb, qt_ps, bqk_cols[:, 0, p:p + 1])
            sink[0] = q_sb

        def emit_kt(p, ch, sink):
            sl = slice(ch * 512, (ch + 1) * 512)
            kt_ps = ppp.tile([128, 512], F32, name=f"kt_ps{ch}", tag="pp")
            for c in range(DC):
                _mm(nc, kt_ps, wqk_sb[:, p, c, :, 128:256], xpk[:, c, :, sl],
                    start=(c == 0), stop=(c == DC - 1), perf_mode=DR)
            k_sb = qkp.tile([128, 512], MM_DT, name=f"k_sb{ch}", tag=f"k{ch}")
            nc.vector.tensor_scalar_add(k_sb, kt_ps, bqk_cols[:, 1, p:p + 1])
            sink[1][ch] = k_sb

        def emit_proj(p):
            sink = [None, [None, None]]
            emit_qt(p, sink)
            emit_kt(p, 0, sink)
            emit_kt(p, 1, sink)
            return sink

        def emit_att(p, sink, fillers=()):
            q_sb, k_chunks = sink[0], sink[1]
            gi = GROUP_OF_PAIR[p]
            g0 = GROUPS[gi][0]
            v_sb = v_tiles[gi]
            g0_tiles = [g_all[:, 2 * p + sub, :] for sub in range(2)]

            eo_tiles = [
                pep.tile([H + 1, SL], F32, name=f"eo_ps{s}", tag="eo")
                for s in range(2)
            ]

            def emit_eo(ats, c):
                for sub in range(2):
                    i = 2 * p + sub - g0
                    _mm(nc, eo_tiles[sub],
                        v_sb[:, 2 * c:2 * c + 2, i, 0:H + 1], ats[sub],
                        start=(c == 0), stop=(c == KT // 2 - 1),
                        perf_mode=DR)

            # scores: alternate sub-expert row groups (rows 0-63 / 64-127)
            # on consecutive matmuls so disjoint row tiles overlap in the PE;
            # eo trails the exp stream by two c-steps (6 at2 buffers) so the
            # PE never waits on the current exp
            pending = []
            for c in range(KT // 2):
                scs = [scp.tile([128, 2, SL], F32, name=f"sc2_{s}", tag="sc")
                       for s in range(2)]
                for j in range(2):
                    kt = 2 * c + j
                    ksl = slice((kt % 4) * 128, (kt % 4 + 1) * 128)
                    for sub in range(2):
                        po = sub * 64
                        _mm(nc, scs[sub][:, j, :],
                            k_chunks[kt // 4][po:po + 64, ksl],
                            q_sb[po:po + 64, :], start=True, stop=True)
                ats = []
                for sub in range(2):
                    at2 = atp.tile([128, 2, SL], FP8, name="at2", tag="at")
                    nc.scalar.activation(at2, scs[sub], AF.Exp, scale=SCALE)
                    ats.append(at2)
                pending.append((ats, c))
                if len(pending) > 2:
                    emit_eo(*pending.pop(0))
                nf = len(fillers)
                if nf and c >= KT // 2 - nf:
                    fillers[c - (KT // 2 - nf)]()
            while pending:
                emit_eo(*pending.pop(0))

            # gate/sumexp normalization of eoT -> combT[p]
            for sub in range(2):
                eo_ps = eo_tiles[sub]
                s_inv = smp.tile([1, SL], F32, name="s_inv")
                nc.vector.reciprocal(s_inv, eo_ps[H:H + 1, :])
                g_row = smp.tile([1, SL], F32, name="g_row")
                nc.vector.tensor_mul(g_row, s_inv, g0_tiles[sub])
                sc64 = smp.tile([H, SL], F32, name="sc64")
                nc.gpsimd.partition_broadcast(sc64, g_row)
                nc.vector.tensor_mul(
                    combT[p][sub * 64:sub * 64 + 64, :], eo_ps[0:H, :], sc64
                )

        # ---- pipelined schedule: PROJ(p+1) lands before ATT(p) so its
        # q/k copies hide under the previous pair's attention.  V-chains are
        # spread as late-c-slot fillers inside attention instead of serial
        # PE blocks that starve Act: eo(c) only reads key-tiles 2c/2c+1, so
        # group 0's chains 4-7 can fill ATT(0) itself, and each later
        # group's 8 chains spread over the preceding pairs ----
        alloc_vgroup(0)
        for tt in range(4):
            emit_vchain(0, tt)
        finish_gates()
        # 2 chains per pair, maximally smooth: a group's kt-6/7 chains may
        # ride their consumer pair's own early slots because eo(c=3) only
        # reads them in the post-loop drain
        FILLER_PLAN = {
            0: [(0, 4), (0, 5), (0, 6), (0, 7)],
            1: [(1, 0), (1, 1)],
            2: [(1, 2), (1, 3)],
            3: [(1, 4), (1, 5)],
            4: [(1, 6), (1, 7)],
            5: [(2, 0), (2, 1)],
            6: [(2, 2), (2, 3)],
            7: [(2, 4), (2, 5)],
            8: [(2, 6), (2, 7)],
        }
        ALLOC_AT = {1: 1, 5: 2}   # pair -> group tile to allocate first
        proj = {0: emit_proj(0)}
        for p in range(EP):
            nxt = p + 1
            if p in ALLOC_AT:
                alloc_vgroup(ALLOC_AT[p])
            fillers = tuple(
                (lambda g=g, t=t: emit_vchain(g, t))
                for g, t in FILLER_PLAN.get(p, []))
            if nxt < EP:
                proj[nxt] = emit_proj(nxt)
            emit_att(p, proj.pop(p), fillers)

        # ---- out projection: for each token tile, one N=1024 chunk and one
        # N=256 chunk, sharing each combT[ht] stationary back-to-back ----
        OCH = [(0, 512), (512, 1024), (1024, 1280)]
        last_tt = SL // 128 - 1
        for tt in range(SL // 128):
            tsl = slice(tt * 128, (tt + 1) * 128)
            o_sb = io.tile([128, D], F32, name="o_sb", tag="o_sb")
            for ci, (c0, c1) in enumerate(OCH):
                op_ps = ppp.tile([128, c1 - c0], F32, name=f"op{ci}", tag="pp")
                for ht in range(HT):
                    _mm(nc, op_ps, combT[ht][:, tsl], ow_sb[:, ht, c0:c1],
                        start=(ht == 0), stop=(ht == HT - 1))
                nc.vector.tensor_add(o_sb[:, c0:c1], op_ps, ob_bc[:, c0:c1])
                if tt == last_tt:
                    # per-chunk stores so the kernel tail is only the last
                    # 256-wide chunk's copy + DMA
                    nc.sync.dma_start(out=out_d[tsl, c0:c1],
                                      in_=o_sb[:, c0:c1])
            if tt != last_tt:
                nc.sync.dma_start(out=out_d[tsl, :], in_=o_sb)


def declare_tensors(nc):
    xpk_d = nc.dram_tensor("xpk", [128, DC, 2, S], FP8, kind="ExternalInput").ap()
    wqk_d = nc.dram_tensor("wqk", [128, EP, DC, 2, 256], FP8,
                           kind="ExternalInput").ap()
    bqk_d = nc.dram_tensor("bqk", [128, 2, EP], F32, kind="ExternalInput").ap()
    wv_d = nc.dram_tensor("wv", [128, DC, 2, E * H], FP8,
                          kind="ExternalInput").ap()
    bv_d = nc.dram_tensor("bv", [1, E * H], MM_DT, kind="ExternalInput").ap()
    rw_d = nc.dram_tensor("router_w", [128, DC, 2, 64], FP8,
                          kind="ExternalInput").ap()
    rb_d = nc.dram_tensor("router_b", [64], MM_DT, kind="ExternalInput").ap()
    ow_d = nc.dram_tensor("out_w", [D, D], MM_DT, kind="ExternalInput").ap()
    ob_d = nc.dram_tensor("out_b", [D], MM_DT, kind="ExternalInput").ap()
    out_d = nc.dram_tensor("out", [SL, D], F32, kind="ExternalOutput").ap()
    return (xpk_d, wqk_d, bqk_d, wv_d, bv_d, rw_d, rb_d, ow_d, ob_d, out_d)


def build_nc():
    nc = bacc.Bacc("TRN2", target_bir_lowering=False, debug=False,
                   num_devices=NCORES)
    tensors = declare_tensors(nc)
    with tile.TileContext(nc) as tc:
        _emit(tc, *tensors)
    nc.compile()
    return nc


_NC = None


def _get_nc():
    global _NC
    if _NC is None:
        _NC = build_nc()
    return _NC


def _pack_dr(a):
    """[D, N] f32 -> [128, DC, 2, N] fp8e4m3 (DoubleRow contraction layout)."""
    a = np.asarray(a, np.float32)
    n = a.shape[1]
    return np.ascontiguousarray(
        a.reshape(DC, 2, 128, n).transpose(2, 0, 1, 3).astype(NP_F8))


def make_in_maps(x, wqkv, bqkv, router_w, router_b, out_w, out_b):
    x = np.ascontiguousarray(np.asarray(x, np.float32))
    wqkv = np.asarray(wqkv, np.float32)
    bqkv = np.asarray(bqkv, np.float32)
    wq = wqkv[:, :, 0:H].reshape(EP, 2, D, H).transpose(0, 2, 1, 3).reshape(EP, D, 128)
    wk = wqkv[:, :, H:2 * H].reshape(EP, 2, D, H).transpose(0, 2, 1, 3).reshape(EP, D, 128)
    wqk = np.concatenate([wq, wk], axis=-1)  # [EP, D, 256]
    # -> [128, EP, DC, 2, 256] fp8
    wqk_pk = np.ascontiguousarray(
        wqk.reshape(EP, DC, 2, 128, 256).transpose(3, 0, 1, 2, 4).astype(NP_F8))
    bq = bqkv[:, 0:H].reshape(EP, 128)     # row layout (e0h0-63, e1h0-63)
    bk = bqkv[:, H:2 * H].reshape(EP, 128)
    # f32 per-partition bias columns for tensor_scalar_add: [128, {q,k}, EP]
    bqk_cols = np.ascontiguousarray(
        np.stack([bq.T, bk.T], axis=1).astype(np.float32))
    wv = wqkv[:, :, 2 * H:3 * H].transpose(1, 0, 2).reshape(D, E * H)
    bv = bqkv[:, 2 * H:3 * H].reshape(1, E * H)

    def _c(a):
        return np.ascontiguousarray(np.asarray(a, np.float32).astype(NP_MM))
    shared = {
        "wqk": wqk_pk, "bqk": bqk_cols, "wv": _pack_dr(wv), "bv": _c(bv),
        "router_w": _pack_dr(np.pad(np.asarray(router_w, np.float32),
                                    ((0, 0), (0, 64 - E)))),
        "router_b": _c(np.pad(np.asarray(router_b, np.float32), (0, 64 - E))),
        "out_w": _c(out_w), "out_b": _c(out_b),
    }
    in_maps = []
    for c in range(NCORES):
        b, half = c // 2, c % 2
        xb = x[b]
        if half == 0:
            x_ctx = xb
        else:
            x_ctx = np.concatenate([xb[SL:], xb[:SL]], axis=0)
        in_maps.append({"xpk": _pack_dr(x_ctx.T), **shared})
    return in_maps


def gather_out(results):
    out = np.empty((B, S, D), np.float32)
    for c in range(NCORES):
        b, half = c // 2, c % 2
        out[b, half * SL:(half + 1) * SL] = results[c]["out"]
    return out


def kernel(x, wqkv, bqkv, router_w, router_b, out_w, out_b):
    nc = _get_nc()
    in_maps = make_in_maps(x, wqkv, bqkv, router_w, router_b, out_w, out_b)
    res = run_bass_kernel_spmd(nc, in_maps, core_ids=list(range(NCORES)))
    return gather_out(res.results)


# revision 27
# speedup vs baseline: 2.5899x; 1.0019x over previous
"""MoE-attention kernel for 8 Trainium2 NeuronCores.

Sharding: token-parallel. Core c handles sequence b = c//2, query-token half
half = c%2 (512 query tokens). Each core computes all 20 experts for its
query tokens; K/V context is the full 1024-token sequence, fed with the
local half FIRST (attention is permutation-invariant over key positions).
No collectives: out_proj partial sums are avoided by giving every core the
full feature dim for its own query tokens.

v4 layout strategy (fp8 DoubleRow matmuls for projections + attn*v):
  x, wv, wqk, rw  host-packed fp8e4m3 as [128, 5, 2, N]: contraction d-dim
                  split into 5 chunks of 256 = (2 planes x 128 partitions),
                  consumed by DoubleRow matmuls (measured ~1.9x bf16 FLOP/s)
  biases          bf16, added via regular K=1 matmuls into the same PSUM group
  q/k             PSUM f32 -> SBUF bf16 (scores stay bf16: contraction is only
                  64, served by PE quadrant pairs at full rate; fp8 DR cannot
                  beat that shape)
  scoresT [128kt, 2, 512qt]  2 key-tiles per PSUM tile -> one big exp
  attnT = exp(scoresT*scale) emitted as fp8 (no max-subtraction: scores are
                  bounded, exp < ~50 fits fp8e4m3 range 448)
  v       [128t, kt, e, 65] fp8 with ones column -> sumexp row
  eoT     [65h, 512qt] = DoubleRow over (v 2-key-tile planes, attn planes)
  combT   10 per-ht tiles [128h', 512qt] = eoT * gate/sumexp   (bf16)
  out     [512t, 1280] = combT.T @ out_w + out_b, all bf16 (fp8 out_proj
                  fails the accuracy budget; measured numerically)
  software pipeline: PROJ(p+1) emitted before ATT(p), so the projection
  copies always hide under the previous pair's attention; within ATT,
  exp(unit u) on Act overlaps eo-matmuls(u-1) on PE
"""

import numpy as np

import concourse.bass as bass
import concourse.mybir as mybir
import concourse.tile as tile
from concourse import bacc
from concourse.bass_utils import run_bass_kernel_spmd

F32 = mybir.dt.float32
BF16 = mybir.dt.bfloat16
FP8 = mybir.dt.float8e4
DR = mybir.MatmulPerfMode.DoubleRow
MM_DT = BF16
try:
    import ml_dtypes as _mld
    NP_MM = np.dtype(_mld.bfloat16)
    NP_F8 = np.dtype(_mld.float8_e4m3)
except Exception as e:  # pragma: no cover
    raise RuntimeError("ml_dtypes required for bf16/fp8 host packing") from e
AF = mybir.ActivationFunctionType

B = 4
S = 1024          # sequence length (full context per core)
D = 1280          # d_model
E = 20            # experts
EP = E // 2       # expert pairs
H = 64            # head dim
SL = 512          # local query tokens per core
DC = 5            # d-model DoubleRow chunks (5 x (2 x 128) = 1280)
DT = D // 128     # 10 d-tiles (out_proj contraction)
HT = D // 128     # 10 h'-tiles
KT = S // 128     # 8 key tiles
SCALE = float(H) ** -0.5
NCORES = 8
GROUPS = [(0, 8), (8, 16), (16, 20)]
GROUP_OF_PAIR = {p: gi for gi, (g0, g1) in enumerate(GROUPS)
                 for p in range(g0 // 2, g1 // 2)}
PAIR_STARTS_GROUP = {g0 // 2: gi for gi, (g0, g1) in enumerate(GROUPS)}


def _mm(nc, out, lhsT, rhs, **kw):
    nc.tensor.matmul(out, lhsT, rhs, **kw)


def _emit(tc, xpk_d, wqk_d, bqk_d, wv_d, bv_d, rw_d, rb_d, ow_d, ob_d, out_d):
    nc = tc.nc
    with (
        tc.tile_pool(name="const", bufs=1) as const,
        tc.tile_pool(name="io", bufs=2) as io,
        tc.tile_pool(name="vp", bufs=2) as vp,
        tc.tile_pool(name="qk", bufs=2) as qkp,
        tc.tile_pool(name="at", bufs=6) as atp,
        tc.tile_pool(name="sm", bufs=2) as smp,
        tc.tile_pool(name="scp", bufs=2, space="PSUM") as scp,
        tc.tile_pool(name="ppp", bufs=2, space="PSUM") as ppp,
        tc.tile_pool(name="pe", bufs=2, space="PSUM") as pep,
    ):
        # ---- constants + all weights up-front (few large DMAs) ----
        ones_f32 = const.tile([128, 512], F32, name="ones_f32")
        nc.gpsimd.memset(ones_f32, 1.0)
        ones_row = const.tile([1, S], MM_DT, name="ones_row")
        nc.vector.tensor_copy(ones_row[:, 0:512], ones_f32[0:1, :])
        nc.vector.tensor_copy(ones_row[:, 512:1024], ones_f32[0:1, :])
        ones20 = const.tile([E, 1], MM_DT, name="ones20")
        nc.vector.tensor_copy(ones20, ones_f32[0:E, 0:1])

        # router weights first (tiny), then x packed as three c-chunks so the
        # router chain starts as soon as the first chunk lands
        # router weights padded to 64 output cols: DoubleRow with M=20
        # (tile col 32) fails the walrus ISA check; M=64 passes
        rw_sb = const.tile([128, DC, 2, 64], FP8, name="rw_sb")
        nc.sync.dma_start(out=rw_sb, in_=rw_d)
        rb_sb = const.tile([1, 64], MM_DT, name="rb_sb")
        nc.sync.dma_start(out=rb_sb, in_=rb_d[None, :])

        # DMA issue queues: sync (SP) for early/critical transfers, gpsimd
        # (Pool, ~free dispatch) for the bulk tail. Act issues none: its
        # queue is the exp bottleneck.
        xpk = const.tile([128, DC, 2, S], FP8, name="xpk")
        nc.sync.dma_start(out=xpk[:, 0:1], in_=xpk_d[:, 0:1])
        nc.gpsimd.dma_start(out=xpk[:, 1:3], in_=xpk_d[:, 1:3])
        nc.sync.dma_start(out=xpk[:, 3:5], in_=xpk_d[:, 3:5])

        wv_sb = const.tile([128, DC, 2, E * H], FP8, name="wv_sb")
        nc.gpsimd.dma_start(out=wv_sb[:, :, :, 0:8 * H],
                            in_=wv_d[:, :, :, 0:8 * H])
        nc.sync.dma_start(out=wv_sb[:, :, :, 8 * H:], in_=wv_d[:, :, :, 8 * H:])
        bv_row = const.tile([1, E * H], MM_DT, name="bv_row")
        nc.sync.dma_start(out=bv_row, in_=bv_d)

        wqk_sb = const.tile([128, EP, DC, 2, 256], FP8, name="wqk_sb")
        nc.sync.dma_start(out=wqk_sb[:, 0:2], in_=wqk_d[:, 0:2])
        nc.gpsimd.dma_start(out=wqk_sb[:, 2:6], in_=wqk_d[:, 2:6])
        nc.gpsimd.dma_start(out=wqk_sb[:, 6:], in_=wqk_d[:, 6:])
        bqk_cols = const.tile([128, 2, EP], F32, name="bqk_cols")
        nc.sync.dma_start(out=bqk_cols, in_=bqk_d)

        ow_sb = const.tile([128, DT, D], MM_DT, name="ow_sb")
        ow_in = ow_d.rearrange("(t p) n -> p t n", p=128)
        nc.gpsimd.dma_start(out=ow_sb[:, 0:DT // 2], in_=ow_in[:, 0:DT // 2])
        nc.gpsimd.dma_start(out=ow_sb[:, DT // 2:], in_=ow_in[:, DT // 2:])
        ob_sb = const.tile([1, D], MM_DT, name="ob_sb")
        nc.sync.dma_start(out=ob_sb, in_=ob_d[None, :])

        # f32 broadcast bias planes: the V and out-proj bias matmuls (24+12
        # K=1 matmuls, ~310ns PE each) become part of the existing DVE
        # PSUM->SBUF copies via tensor_add against these
        bv32 = const.tile([1, E * H], F32, name="bv32")
        nc.vector.tensor_copy(bv32, bv_row)
        bv_bc = const.tile([128, E * H], F32, name="bv_bc")
        nc.gpsimd.partition_broadcast(bv_bc, bv32)
        ob32 = const.tile([1, D], F32, name="ob32")
        nc.vector.tensor_copy(ob32, ob_sb)
        ob_bc = const.tile([128, D], F32, name="ob_bc")
        nc.gpsimd.partition_broadcast(ob_bc, ob32)

        # per-ht combined-feature tiles (separate so out_proj's ht chain only
        # waits on the normalize that wrote that ht)
        combT = [const.tile([128, SL], MM_DT, name=f"combT{h}")
                 for h in range(HT)]

        # ---- router logits (gates finished after group-0 V so the PE
        # never waits on the Act queue's DMA issues) ----
        exp_router = const.tile([E, SL], MM_DT, name="exp_router")
        gates_sb = const.tile([E, SL], F32, name="gates_sb")
        inv_rsum = const.tile([1, SL], F32, name="inv_rsum")
        inv_rep = const.tile([E, SL], F32, name="inv_rep")

        rt_ps = ppp.tile([64, SL], F32, name="rt_ps", tag="pp")
        for c in range(DC):
            _mm(nc, rt_ps, rw_sb[:, c], xpk[:, c, :, 0:SL],
                start=(c == 0), stop=False, perf_mode=DR)
        _mm(nc, rt_ps, rb_sb, ones_row[:, 0:SL], start=False, stop=True)
        nc.scalar.activation(exp_router, rt_ps[0:E, :], AF.Exp)

        g_all = const.tile([1, E, SL], F32, name="g_all")

        def finish_gates():
            rs_ps = ppp.tile([1, SL], F32, name="rs_ps", tag="pp")
            _mm(nc, rs_ps, ones20, exp_router, start=True, stop=True)
            nc.vector.reciprocal(inv_rsum, rs_ps)
            nc.gpsimd.partition_broadcast(inv_rep, inv_rsum)
            nc.vector.tensor_mul(gates_sb, exp_router, inv_rep)
            # all 20 gate rows onto partition 0 in ONE transfer; per-pair
            # normalizes then just slice g_all (the per-pair row DMAs used
            # to arrive late and stall the eo-PSUM handoff)
            nc.sync.dma_start(out=g_all, in_=gates_sb[:, :])

        v_tiles = {}

        def alloc_vgroup(gi):
            g0, g1 = GROUPS[gi]
            gsz = g1 - g0
            # row width padded 65 -> 68: dual-fp8 Ldweights requires the
            # plane (kt) byte-stride to be a multiple of 16 (gsz*68 = 544/272)
            v_sb = vp.tile([128, KT, gsz, 68], FP8, name="v_sb", tag="vg")
            v_tiles[gi] = v_sb
            nc.vector.tensor_copy(
                v_sb[:, :, :, H],
                ones_f32[:, 0:KT * gsz].rearrange("p (a b) -> p a b", a=KT),
            )

        def emit_vchain(gi, tt):
            g0, g1 = GROUPS[gi]
            gsz = g1 - g0
            v_sb = v_tiles[gi]
            v_ps = ppp.tile([128, gsz * H], F32, name="v_ps", tag="pp")
            for c in range(DC):
                _mm(nc, v_ps, xpk[:, c, :, tt * 128:(tt + 1) * 128],
                    wv_sb[:, c, :, g0 * H:g1 * H],
                    start=(c == 0), stop=(c == DC - 1), perf_mode=DR)
            nc.vector.tensor_add(
                v_sb[:, tt, :, 0:H],
                v_ps.rearrange("p (e h) -> p e h", e=gsz),
                bv_bc[:, g0 * H:g1 * H].rearrange("p (e h) -> p e h", e=gsz),
            )

        def emit_qt(p, sink):
            qt_ps = ppp.tile([128, SL], F32, name="qt_ps", tag="pp")
            for c in range(DC):
                _mm(nc, qt_ps, wqk_sb[:, p, c, :, 0:128], xpk[:, c, :, 0:SL],
                    start=(c == 0), stop=(c == DC - 1), perf_mode=DR)
            q_sb = qkp.tile([128, SL], MM_DT, name="q_sb", tag="q")
            nc.vector.tensor_scalar_add(q_sb, qt_ps, bqk_cols[:, 0, p:p + 1])
            sink[0] = q_sb

        def emit_kt(p, ch, sink):
            sl = slice(ch * 512, (ch + 1) * 512)
            kt_ps = ppp.tile([128, 512], F32, name=f"kt_ps{ch}", tag="pp")
            for c in range(DC):
                _mm(nc, kt_ps, wqk_sb[:, p, c, :, 128:256], xpk[:, c, :, sl],
                    start=(c == 0), stop=(c == DC - 1), perf_mode=DR)
            k_sb = qkp.tile([128, 512], MM_DT, name=f"k_sb{ch}", tag=f"k{ch}")
            nc.vector.tensor_scalar_add(k_sb, kt_ps, bqk_cols[:, 1, p:p + 1])
            sink[1][ch] = k_sb

        def emit_proj(p):
            sink = [None, [None, None]]
            emit_qt(p, sink)
            emit_kt(p, 0, sink)
            emit_kt(p, 1, sink)
            return sink

        def emit_att(p, sink, fillers=()):
            q_sb, k_chunks = sink[0], sink[1]
            gi = GROUP_OF_PAIR[p]
            g0 = GROUPS[gi][0]
            v_sb = v_tiles[gi]
            g0_tiles = [g_all[:, 2 * p + sub, :] for sub in range(2)]

            eo_tiles = [
                pep.tile([H + 1, SL], F32, name=f"eo_ps{s}", tag="eo")
                for s in range(2)
            ]

            def emit_eo(ats, c):
                for sub in range(2):
                    i = 2 * p + sub - g0
                    _mm(nc, eo_tiles[sub],
                        v_sb[:, 2 * c:2 * c + 2, i, 0:H + 1], ats[sub],
                        start=(c == 0), stop=(c == KT // 2 - 1),
                        perf_mode=DR)

            # scores: alternate sub-expert row groups (rows 0-63 / 64-127)
            # on consecutive matmuls so disjoint row tiles overlap in the PE;
            # eo trails the exp stream by two c-steps (6 at2 buffers) so the
            # PE never waits on the current exp
            pending = []
            for c in range(KT // 2):
                scs = [scp.tile([128, 2, SL], F32, name=f"sc2_{s}", tag="sc")
                       for s in range(2)]
                for j in range(2):
                    kt = 2 * c + j
                    ksl = slice((kt % 4) * 128, (kt % 4 + 1) * 128)
                    for sub in range(2):
                        po = sub * 64
                        _mm(nc, scs[sub][:, j, :],
                            k_chunks[kt // 4][po:po + 64, ksl],
                            q_sb[po:po + 64, :], start=True, stop=True)
                ats = []
                for sub in range(2):
                    at2 = atp.tile([128, 2, SL], FP8, name="at2", tag="at")
                    nc.scalar.activation(at2, scs[sub], AF.Exp, scale=SCALE)
                    ats.append(at2)
                pending.append((ats, c))
                if len(pending) > 2:
                    emit_eo(*pending.pop(0))
                nf = len(fillers)
                if nf and c >= KT // 2 - nf:
                    fillers[c - (KT // 2 - nf)]()
            while pending:
                emit_eo(*pending.pop(0))

            # gate/sumexp normalization of eoT -> combT[p]
            for sub in range(2):
                eo_ps = eo_tiles[sub]
                s_inv = smp.tile([1, SL], F32, name="s_inv")
                nc.vector.reciprocal(s_inv, eo_ps[H:H + 1, :])
                g_row = smp.tile([1, SL], F32, name="g_row")
                nc.vector.tensor_mul(g_row, s_inv, g0_tiles[sub])
                sc64 = smp.tile([H, SL], F32, name="sc64")
                nc.gpsimd.partition_broadcast(sc64, g_row)
                nc.vector.tensor_mul(
                    combT[p][sub * 64:sub * 64 + 64, :], eo_ps[0:H, :], sc64
                )

        # ---- pipelined schedule: PROJ(p+1) lands before ATT(p) so its
        # q/k copies hide under the previous pair's attention.  V-chains are
        # spread as late-c-slot fillers inside attention instead of serial
        # PE blocks that starve Act: eo(c) only reads key-tiles 2c/2c+1, so
        # group 0's chains 4-7 can fill ATT(0) itself, and each later
        # group's 8 chains spread over the preceding pairs ----
        alloc_vgroup(0)
        for tt in range(4):
            emit_vchain(0, tt)
        finish_gates()
        # 2 chains per pair, maximally smooth: a group's kt-6/7 chains may
        # ride their consumer pair's own early slots because eo(c=3) only
        # reads them in the post-loop drain
        FILLER_PLAN = {
            0: [(0, 4), (0, 5), (0, 6), (0, 7)],
            1: [(1, 0), (1, 1)],
            2: [(1, 2), (1, 3)],
            3: [(1, 4), (1, 5)],
            4: [(1, 6), (1, 7)],
            5: [(2, 0), (2, 1)],
            6: [(2, 2), (2, 3)],
            7: [(2, 4), (2, 5)],
            8: [(2, 6), (2, 7)],
        }
        ALLOC_AT = {1: 1, 5: 2}   # pair -> group tile to allocate first
        proj = {0: emit_proj(0)}
        for p in range(EP):
            nxt = p + 1
            if p in ALLOC_AT:
                alloc_vgroup(ALLOC_AT[p])
            fillers = tuple(
                (lambda g=g, t=t: emit_vchain(g, t))
                for g, t in FILLER_PLAN.get(p, []))
            if nxt < EP:
                proj[nxt] = emit_proj(nxt)
            emit_att(p, proj.pop(p), fillers)

        # ---- out projection: for each token tile, one N=1024 chunk and one
        # N=256 chunk, sharing each combT[ht] stationary back-to-back ----
        OCH = [(0, 512), (512, 1024), (1024, 1280)]
        last_tt = SL // 128 - 1
        for tt in range(SL // 128):
            tsl = slice(tt * 128, (tt + 1) * 128)
            o_sb = io.tile([128, D], F32, name="o_sb", tag="o_sb")
            for ci, (c0, c1) in enumerate(OCH):
                op_ps = ppp.tile([128, c1 - c0], F32, name=f"op{ci}", tag="pp")
                for ht in range(HT):
                    _mm(nc, op_ps, combT[ht][:, tsl], ow_sb[:, ht, c0:c1],
                        start=(ht == 0), stop=(ht == HT - 1))
                nc.vector.tensor_add(o_sb[:, c0:c1], op_ps, ob_bc[:, c0:c1])
                if tt == last_tt:
                    # per-chunk stores so the kernel tail is only the last
                    # 256-wide chunk's copy + DMA
                    nc.sync.dma_start(out=out_d[tsl, c0:c1],
                                      in_=o_sb[:, c0:c1])
            if tt != last_tt:
                nc.sync.dma_start(out=out_d[tsl, :], in_=o_sb)


def declare_tensors(nc):
    xpk_d = nc.dram_tensor("xpk", [128, DC, 2, S], FP8, kind="ExternalInput").ap()
    wqk_d = nc.dram_tensor("wqk", [128, EP, DC, 2, 256], FP8,
                           kind="ExternalInput").ap()
    bqk_d = nc.dram_tensor("bqk", [128, 2, EP], F32, kind="ExternalInput").ap()
    wv_d = nc.dram_tensor("wv", [128, DC, 2, E * H], FP8,
                          kind="ExternalInput").ap()
    bv_d = nc.dram_tensor("bv", [1, E * H], MM_DT, kind="ExternalInput").ap()
    rw_d = nc.dram_tensor("router_w", [128, DC, 2, 64], FP8,
                          kind="ExternalInput").ap()
    rb_d = nc.dram_tensor("router_b", [64], MM_DT, kind="ExternalInput").ap()
    ow_d = nc.dram_tensor("out_w", [D, D], MM_DT, kind="ExternalInput").ap()
    ob_d = nc.dram_tensor("out_b", [D], MM_DT, kind="ExternalInput").ap()
    out_d = nc.dram_tensor("out", [SL, D], F32, kind="ExternalOutput").ap()
    return (xpk_d, wqk_d, bqk_d, wv_d, bv_d, rw_d, rb_d, ow_d, ob_d, out_d)


def build_nc():
    nc = bacc.Bacc("TRN2", target_bir_lowering=False, debug=False,
                   num_devices=NCORES)
    tensors = declare_tensors(nc)
    with tile.TileContext(nc) as tc:
        _emit(tc, *tensors)
    nc.compile()
    return nc


_NC = None


def _get_nc():
    global _NC
    if _NC is None:
        _NC = build_nc()
    return _NC


def _pack_dr(a):
    """[D, N] f32 -> [128, DC, 2, N] fp8e4m3 (DoubleRow contraction layout)."""
    a = np.asarray(a, np.float32)
    n = a.shape[1]
    return np.ascontiguousarray(
        a.reshape(DC, 2, 128, n).transpose(2, 0, 1, 3).astype(NP_F8))


def make_in_maps(x, wqkv, bqkv, router_w, router_b, out_w, out_b):
    x = np.ascontiguousarray(np.asarray(x, np.float32))
    wqkv = np.asarray(wqkv, np.float32)
    bqkv = np.asarray(bqkv, np.float32)
    wq = wqkv[:, :, 0:H].reshape(EP, 2, D, H).transpose(0, 2, 1, 3).reshape(EP, D, 128)
    wk = wqkv[:, :, H:2 * H].reshape(EP, 2, D, H).transpose(0, 2, 1, 3).reshape(EP, D, 128)
    wqk = np.concatenate([wq, wk], axis=-1)  # [EP, D, 256]
    # -> [128, EP, DC, 2, 256] fp8
    wqk_pk = np.ascontiguousarray(
        wqk.reshape(EP, DC, 2, 128, 256).transpose(3, 0, 1, 2, 4).astype(NP_F8))
    bq = bqkv[:, 0:H].reshape(EP, 128)     # row layout (e0h0-63, e1h0-63)
    bk = bqkv[:, H:2 * H].reshape(EP, 128)
    # f32 per-partition bias columns for tensor_scalar_add: [128, {q,k}, EP]
    bqk_cols = np.ascontiguousarray(
        np.stack([bq.T, bk.T], axis=1).astype(np.float32))
    wv = wqkv[:, :, 2 * H:3 * H].transpose(1, 0, 2).reshape(D, E * H)
    bv = bqkv[:, 2 * H:3 * H].reshape(1, E * H)

    def _c(a):
        return np.ascontiguousarray(np.asarray(a, np.float32).astype(NP_MM))
    shared = {
        "wqk": wqk_pk, "bqk": bqk_cols, "wv": _pack_dr(wv), "bv": _c(bv),
        "router_w": _pack_dr(np.pad(np.asarray(router_w, np.float32),
                                    ((0, 0), (0, 64 - E)))),
        "router_b": _c(np.pad(np.asarray(router_b, np.float32), (0, 64 - E))),
        "out_w": _c(out_w), "out_b": _c(out_b),
    }
    in_maps = []
    for c in range(NCORES):
        b, half = c // 2, c % 2
        xb = x[b]
        if half == 0:
            x_ctx = xb
        else:
            x_ctx = np.concatenate([xb[SL:], xb[:SL]], axis=0)
        in_maps.append({"xpk": _pack_dr(x_ctx.T), **shared})
    return in_maps


def gather_out(results):
    out = np.empty((B, S, D), np.float32)
    for c in range(NCORES):
        b, half = c // 2, c % 2
        out[b, half * SL:(half + 1) * SL] = results[c]["out"]
    return out


def kernel(x, wqkv, bqkv, router_w, router_b, out_w, out_b):
    nc = _get_nc()
    in_maps = make_in_maps(x, wqkv, bqkv, router_w, router_b, out_w, out_b)
    res = run_bass_kernel_spmd(nc, in_maps, core_ids=list(range(NCORES)))
    return gather_out(res.results)
